# revision 3
# baseline (speedup 1.0000x reference)
"""Trainium2 Bass kernel for nn_BaselineBlock_SCA_Modulated — v2.

Sharding: 8 cores = 2 batch x 4 D-slabs of 16 planes. Halo planes staged
host-side (zeros at global D edges) so all cores run one SPMD program.

v2 changes vs baseline:
- Conv chain in fp8 (e4m3) with DoubleRow matmuls: 27 taps + boundary corr
  in 11 matmuls / 7 N-units per 512-chunk (vs 16 bf16 matmuls).
- Gelu output (xg) stays resident in SBUF as fp8 (no DRAM round trip).
- LN stats via bn_stats; dual-shift ring copies via uint16-bitcast 2x copies.
- Residual adds done on PE (identity f32r matmuls into PSUM) with Act
  readout, instead of DVE affine ops.
- DMA queues split: loads on SP, stores on Pool.
"""
import numpy as np
import ml_dtypes

C, DW, SD = 64, 128, 512
D, H, W = 64, 64, 64
NPL = 16              # output planes per core
NHALO = NPL + 2       # input planes incl halo
PW = 68               # padded row width (2 left pad + 64 + 2 right pad)
NPR = 66              # padded row count (1 top + 64 + 1 bottom)
PSZ = NPR * PW        # padded plane size (even)
HWC = H * W           # 4096
NCH = 32              # 128-position chunks per plane
EPS = 1e-6
S_W = 256.0           # fp8 weight prescale
bf = ml_dtypes.bfloat16
f8 = ml_dtypes.float8_e4m3fn

# (kind, kd, kh): kind 0 = P-read (kw0 lower / kw1 upper, col 1),
# kind 1 = S-read (kw2 lower, zero upper, col 3)
TAPS = [(0, kd, kh) for kd in range(3) for kh in range(3)] + \
       [(1, kd, kh) for kd in range(3) for kh in range(3)]
DRPAIRS = [(TAPS[2 * i], TAPS[2 * i + 1]) for i in range(9)]

_CACHE = {}


def _build():
    import concourse.bacc as bacc
    import concourse.mybir as mybir
    import concourse.tile as tile
    from concourse.ap import AP
    from concourse.mybir import ActivationFunctionType as AF, AluOpType as ALU

    BF = mybir.dt.bfloat16
    F32 = mybir.dt.float32
    F32R = mybir.dt.float32r
    FP8 = mybir.dt.float8e4
    U16 = mybir.dt.uint16
    AX = mybir.AxisListType
    DR = mybir.MatmulPerfMode.DoubleRow

    nc = bacc.Bacc("TRN2", target_bir_lowering=False, debug=False, num_devices=8)

    dram = {}
    def din(name, shape, dt=BF):
        dram[name] = nc.dram_tensor(name, shape, dt, kind="ExternalInput")
        return dram[name]

    inp_t = din("inp_t", [NHALO, C, HWC], BF)
    inp_f = din("inp_f", [NPL, C, HWC], F32R)
    blob8_i = din("blob8", [128, 8064], FP8)
    blobb_i = din("blobb", [128, 512], BF)
    blobf_i = din("blobf", [128, 70], F32)
    i64f_i = din("i64f", [64, 64], F32R)
    out_d = nc.dram_tensor("out", [NPL, C, HWC], F32, kind="ExternalOutput")

    cc_a = nc.dram_tensor("cc_a", [128, 1], F32)
    cc_b = nc.dram_tensor("cc_b", [128, 1], F32)

    from contextlib import ExitStack
    with tile.TileContext(nc) as tc, ExitStack() as stk:
        cpool = stk.enter_context(tc.tile_pool(name="const", bufs=1))
        sm = stk.enter_context(tc.tile_pool(name="small", bufs=2))
        p1stk = ExitStack()
        wp1 = p1stk.enter_context(tc.tile_pool(name="p1", bufs=2))
        rpool = p1stk.enter_context(tc.tile_pool(name="ring", bufs=1))
        psC = p1stk.enter_context(tc.tile_pool(name="psC", bufs=3,
                                               space="PSUM"))

        def const(name, shape, dt):
            t = cpool.tile(shape, dt, tag=name, name=name)
            nc.sync.dma_start(t[:], dram[name][:])
            return t

        t8 = const("blob8", [128, 8064], FP8)
        tb = const("blobb", [128, 512], BF)
        tf = const("blobf", [128, 70], F32)
        i64t = const("i64f", [64, 64], F32R)
        wPS = t8[:, 0:2304].rearrange("p (q t m) -> p q t m", q=9, t=2)
        i128f8 = t8[:, 2304:2432]
        wc = t8[0:9, 2432:6528].rearrange("p (d t m) -> p d t m", d=16, t=2)
        ind = t8[0:9, 6528:8064].rearrange("p (a b) -> p a b", a=3)
        w3T = tb[:, 0:64]
        scawT = tb[:, 64:192]
        w4T = tb[0:64, 192:320]
        w5T = tb[:, 320:384]
        i128 = tb[:, 384:512]
        sd = tf[:, 0:1]
        modb = tf[:, 1:2]
        scab = tf[:, 2:3]
        b3beta = tf[0:64, 3:4]
        b4 = tf[:, 4:5]
        b5g = tf[0:64, 5:6]
        b35 = tf[0:64, 6:7]
        i64f = i64t[:]

        pools = cpool.tile([128, NPL * 8], F32, tag="pools")
        w3Tp = cpool.tile([128, 64], BF, tag="w3Tp")
        # persistent ring (4 slots in ONE tensor for cross-slot DoubleRow
        # k-tile reads). fp8 values live at byte stride 2 (the fp8 PE
        # transpose writes with element step 2), so the ring is a U16 tile
        # whose low... each u16 cell holds one fp8 value; u16 copies move it.
        ring = rpool.tile([128, 4 * PSZ], U16, tag="ring", name="ring")
        xga = cpool.tile([128, NPL * HWC], FP8, tag="xga", name="xga")
        nc.gpsimd.memset(ring[:], 0)

        rAP = ring[:]
        ring_pstride = rAP.ap[0][0]
        r8 = rAP.bitcast(FP8)

        def rd(parts, offv, dims):
            """fp8 AP into the ring; offv/dims in VALUE units (1 value =
            2 bytes = 2 fp8 elements; strides passed here get doubled)."""
            return AP(r8.tensor, r8.offset + 2 * offv,
                      [[r8.ap[0][0], parts]] + [[2 * s, n] for s, n in dims])

        # ---------------- PASS 1 ----------------
        def ln_stats(xT, sfx, pool):
            """per-(partition,chunk) LN stats over the 64-ch innermost dim.
            Returns (rv, mrv) [128, NCH] f32."""
            sq = pool.tile([128, NCH, 64], BF, tag="sq" + sfx, bufs=1)
            nc.vector.tensor_mul(sq[:], xT[:], xT[:])
            msum = sm.tile([128, NCH], BF, tag="msum" + sfx)
            qsum = sm.tile([128, NCH], BF, tag="qsum" + sfx)
            with nc.allow_low_precision(reason="stat sums; dve accumulates "
                                        "fp32 internally, bf16 round-off is "
                                        "far below the fp8 conv noise"):
                nc.vector.tensor_reduce(msum[:], xT[:], axis=AX.X, op=ALU.add)
                nc.vector.tensor_reduce(qsum[:], sq[:], axis=AX.X, op=ALU.add)
            t1v = sm.tile([128, NCH], F32, tag="t1v" + sfx)
            nc.vector.tensor_mul(t1v[:], msum[:], msum[:])
            t3v = sm.tile([128, NCH], F32, tag="t3v" + sfx)
            nc.vector.tensor_scalar_mul(t3v[:], qsum[:], 1.0 / 63.0)
            var = sm.tile([128, NCH], F32, tag="var" + sfx)
            nc.vector.scalar_tensor_tensor(
                var[:], t1v[:], -1.0 / (64.0 * 63.0), t3v[:],
                op0=ALU.mult, op1=ALU.add)
            # rsqrt(var) via Newton with r0 = 2/(1+v): a global underestimate
            # of rsqrt (AM-GM), so iteration converges monotonically; keeps
            # ACT parked on the gelu table set. eps (1e-6 on std ~1) is far
            # below fp8/bf16 noise and is dropped.
            sv = sm.tile([128, NCH], F32, tag="sv" + sfx)
            nc.vector.tensor_scalar(sv[:], var[:], 0.5, 0.5,
                                    op0=ALU.mult, op1=ALU.add)
            rv = sm.tile([128, NCH], F32, tag="rv" + sfx)
            nc.vector.reciprocal(rv[:], sv[:])
            tq = sm.tile([128, NCH], F32, tag="tq" + sfx)
            for _ in range(2):
                nc.vector.tensor_mul(tq[:], rv[:], rv[:])
                nc.vector.tensor_mul(tq[:], tq[:], var[:])
                nc.vector.tensor_scalar(tq[:], tq[:], -0.5, 1.5,
                                        op0=ALU.mult, op1=ALU.add)
                nc.vector.tensor_mul(rv[:], rv[:], tq[:])
            mrv = sm.tile([128, NCH], F32, tag="mrv" + sfx)
            nc.vector.scalar_tensor_tensor(
                mrv[:], msum[:], 1.0 / 64.0, rv[:], op0=ALU.mult, op1=ALU.mult)
            return rv, mrv

        xl8s = {}

        def ln1a(p):
            xT = wp1.tile([128, NCH, 64], BF, tag="xT")
            nc.sync.dma_start_transpose(xT[:], inp_t[p])
            rv, mrv = ln_stats(xT, "1", wp1)
            # apply LN (mul in place, sub quantizes to fp8)
            rvv = rv[:].unsqueeze(2).broadcast_to([128, NCH, 64])
            mrvv = mrv[:].unsqueeze(2).broadcast_to([128, NCH, 64])
            nc.vector.tensor_mul(xT[:], xT[:], rvv)
            xl8 = wp1.tile([128, NCH, 64], FP8, tag="xl8")
            nc.vector.tensor_sub(xl8[:], xT[:], mrvv)
            xl8s[p] = xl8

        def ln1b(p):
            slot = p % 4
            xl8 = xl8s.pop(p)
            # transpose to ring layout: fp8 transposes write with element
            # step 2 (hw requirement) into psF; u16 copies move value cells.
            for hf in range(2):
                psF = psC.tile([64, HWC], FP8, tag="trF", bufs=2)
                for g in range(16):
                    dst = psF[:, g * 256:(g + 1) * 256].rearrange(
                        "p (c two) -> p c two", two=2)[:, :, 0]
                    nc.tensor.transpose(dst, xl8[:, 16 * hf + g, :], i128f8)
                srcu = psF[:].bitcast(U16).rearrange(
                    "p (r w) -> p r w", w=64)
                o_lo = slot * PSZ + (1 + 32 * hf) * PW + 2
                dst_lo = AP(rAP.tensor, rAP.offset + o_lo,
                            [[ring_pstride, 64], [PW, 32], [1, 64]])
                nc.scalar.copy(dst_lo, srcu)
            # upper dual band (x shifted +1 value) via one contiguous
            # Pool-issued DMA over the full interior row block
            src_full = AP(rAP.tensor, rAP.offset + slot * PSZ + PW,
                          [[ring_pstride, 64], [1, 64 * PW]])
            dst_full = AP(rAP.tensor,
                          rAP.offset + 64 * ring_pstride + slot * PSZ + PW - 1,
                          [[ring_pstride, 64], [1, 64 * PW]])
            nc.gpsimd.dma_start(dst_full, src_full)

        def aoff(slots, kd, kh, cb):
            return slots[kd] * PSZ + (8 * cb + kh) * PW

        def conv_plane(d):
            slots = [(d + kd) % 4 for kd in range(3)]
            pat = lambda cb: 0 if cb == 0 else (2 if cb == 7 else 1)
            for cb in range(8):
                ps = psC.tile([128, 512], F32, tag="mm")
                first = True
                for q, (t1, t2) in enumerate(DRPAIRS):
                    o1 = aoff(slots, t1[1], t1[2], cb) + 1 + 2 * t1[0]
                    o2 = aoff(slots, t2[1], t2[2], cb) + 1 + 2 * t2[0]
                    rhs = rd(128, o1, [[o2 - o1, 2], [PW, 8], [1, 64]])
                    nc.tensor.matmul(ps[:], wPS[:, q], rhs, start=first,
                                     stop=False, perf_mode=DR,
                                     skip_group_check=True)
                    first = False
                rhs_c = ind[:, pat(cb), :].unsqueeze(1).broadcast_to(
                    [9, 2, 512])
                nc.tensor.matmul(ps[:], wc[:, d], rhs_c, start=False,
                                 stop=True, perf_mode=DR,
                                 skip_group_check=True)
                nc.scalar.activation(
                    xga[:, d * HWC + cb * 512: d * HWC + (cb + 1) * 512],
                    ps[:], AF.Gelu, bias=modb, scale=sd,
                    accum_out=pools[:, d * 8 + cb:d * 8 + cb + 1])

        for r in range(NHALO + 2):
            if r < NHALO:
                ln1a(r)
            if r >= 1 and r - 1 < NHALO:
                ln1b(r - 1)
            if r >= 4:
                conv_plane(r - 4)

        p1stk.close()
        p2 = stk.enter_context(tc.tile_pool(name="p2", bufs=2))
        psB2 = stk.enter_context(tc.tile_pool(name="psB2", bufs=2,
                                              space="PSUM"))

        # ---------------- pooled -> gate ----------------
        pooled = cpool.tile([128, 1], F32, tag="pooled")
        nc.vector.tensor_reduce(pooled[:], pools[:], axis=AX.X, op=ALU.add)
        nc.gpsimd.dma_start(cc_a[:], pooled[:])
        nc.gpsimd.collective_compute(
            "AllReduce", ALU.add,
            replica_groups=[[0, 1, 2, 3], [4, 5, 6, 7]],
            ins=[cc_a[:]], outs=[cc_b[:]])
        pooled2f = cpool.tile([128, 1], F32, tag="pooled2f", name="pooled2f")
        nc.gpsimd.dma_start(pooled2f[:], cc_b[:])
        pooled2 = cpool.tile([128, 1], BF, tag="pooled2", name="pooled2")
        nc.vector.tensor_copy(pooled2[:], pooled2f[:])
        psg = psB2.tile([128, 512], F32, tag="m45", bufs=2)
        nc.tensor.matmul(psg[:, 0:1], scawT, pooled2[:], start=True,
                         stop=True)
        gate = cpool.tile([128, 1], F32, tag="gatev")
        nc.scalar.activation(gate[:], psg[:, 0:1], AF.Identity, bias=scab)
        nc.vector.tensor_scalar_mul(w3Tp[:], w3T, gate[:])

        # ---------------- PASS 2 ----------------
        ys = {}
        xl2s = {}

        def p2a1(d):
            xg_d = xga[:, d * HWC:(d + 1) * HWC]
            y = p2.tile([64, HWC], F32R, tag="y", bufs=2)
            ys[d] = y
            for hf in range(2):
                ifp = p2.tile([64, 2048], F32R, tag="ifp", bufs=2)
                nc.sync.dma_start(ifp[:], inp_f[d][:, hf * 2048:(hf + 1) * 2048])
                for cq in range(4):
                    cb = 4 * hf + cq
                    sl = slice(cb * 512, (cb + 1) * 512)
                    ps3 = psB2.tile([64, 512], F32, tag="m3", bufs=3)
                    nc.tensor.matmul(ps3[:], w3Tp[:], xg_d[:, sl],
                                     start=True, stop=False,
                                     skip_group_check=True)
                    nc.tensor.matmul(
                        ps3[:], i64f, ifp[:, cq * 512:(cq + 1) * 512],
                        start=False, stop=True, skip_group_check=True)
                    nc.scalar.activation(y[:, sl], ps3[:], AF.Identity,
                                         bias=b3beta)

        def p2a2(d):
            y = ys.pop(d)
            # LN2 stats (transposed layout)
            yTs = p2.tile([128, NCH, 64], BF, tag="yTs")
            for qf in range(4):
                psT = psB2.tile([128, 512], F32R, tag="tr")
                for g in range(8):
                    cg = 8 * qf + g
                    nc.tensor.transpose(psT[:, g * 64:(g + 1) * 64],
                                        y[:, cg * 128:(cg + 1) * 128], i64f)
                nc.vector.tensor_copy(
                    yTs[:, 8 * qf:8 * (qf + 1), :],
                    psT[:].rearrange("p (g c) -> p g c", c=64))
            rv, mrv = ln_stats(yTs, "2", p2)
            rvv = rv[:].unsqueeze(2).broadcast_to([128, NCH, 64])
            mrvv = mrv[:].unsqueeze(2).broadcast_to([128, NCH, 64])
            nc.vector.tensor_mul(yTs[:], yTs[:], rvv)
            xl2 = p2.tile([128, NCH, 64], BF, tag="xl2")
            nc.vector.tensor_sub(xl2[:], yTs[:], mrvv)
            xl2s[d] = xl2

        xln2s = {}

        def p2b1(d):
            xl2 = xl2s.pop(d)
            xln2 = p2.tile([64, HWC], BF, tag="xln2")
            for hf in range(2):
                for cq in range(4):
                    psX = psB2.tile([64, 512], BF, tag="x4", bufs=1)
                    for g in range(4):
                        cg = 16 * hf + 4 * cq + g
                        nc.tensor.transpose(psX[:, g * 128:(g + 1) * 128],
                                            xl2[:, cg, :], i128)
                    nc.vector.tensor_copy(
                        xln2[:, hf * 2048 + cq * 512:
                             hf * 2048 + (cq + 1) * 512], psX[:])
            xln2s[d] = xln2

        def p2b2(d):
            # out = inp + beta*pw3 + b3beta + gamma*pw5 + b5gamma: the pw3
            # and inp terms are re-accumulated here (instead of adding the
            # staged y) so y only has to live two pipeline rounds.
            xg_d = xga[:, d * HWC:(d + 1) * HWC]
            xln2 = xln2s.pop(d)
            for hf in range(2):
                if2 = p2.tile([64, 2048], F32R, tag="if2", bufs=2)
                nc.gpsimd.dma_start(if2[:],
                                     inp_f[d][:, hf * 2048:(hf + 1) * 2048])
                outp = p2.tile([64, 2048], F32, tag="outp")
                xg2 = p2.tile([128, 2048], BF, tag="xg2")
                for cq in range(4):
                    cb = 4 * hf + cq
                    sl = slice(cb * 512, (cb + 1) * 512)
                    lsl = slice(cq * 512, (cq + 1) * 512)
                    ps4 = psB2.tile([128, 512], F32, tag="m45", bufs=2)
                    nc.tensor.matmul(ps4[:], w4T, xln2[:, sl],
                                     start=True, stop=True)
                    nc.scalar.activation(xg2[:, lsl], ps4[:], AF.Gelu,
                                         bias=b4)
                    ps5 = psB2.tile([64, 512], F32, tag="m45", bufs=2)
                    nc.tensor.matmul(ps5[:], w5T, xg2[:, lsl],
                                     start=True, stop=False,
                                     skip_group_check=True)
                    nc.tensor.matmul(ps5[:], w3Tp[:], xg_d[:, sl],
                                     start=False, stop=False,
                                     skip_group_check=True)
                    nc.tensor.matmul(ps5[:], i64f,
                                     if2[:, cq * 512:(cq + 1) * 512],
                                     start=False, stop=True,
                                     skip_group_check=True)
                    nc.scalar.activation(outp[:, lsl], ps5[:], AF.Identity,
                                         bias=b35)
                nc.gpsimd.dma_start(
                    out_d[d][:, hf * 2048:(hf + 1) * 2048], outp[:])

        for r in range(NPL + 3):
            if r < NPL:
                p2a1(r)
            if r >= 1 and r - 1 < NPL:
                p2a2(r - 1)
            if r >= 2 and r - 2 < NPL:
                p2b1(r - 2)
            if r >= 3:
                p2b2(r - 3)

    nc.compile()
    return nc


def _host_prep(inputs):
    inp = np.asarray(inputs["inp"], np.float32)
    style = np.asarray(inputs["style_vector"], np.float32)
    w1 = np.asarray(inputs["w1"], np.float32)
    b1 = np.asarray(inputs["b1"], np.float32)
    mod_w = np.asarray(inputs["mod_w"], np.float32)
    mod_b = np.asarray(inputs["mod_b"], np.float32)
    style_w = np.asarray(inputs["style_w"], np.float32)
    style_b = np.asarray(inputs["style_b"], np.float32)
    sca_w = np.asarray(inputs["sca_w"], np.float32)
    sca_b = np.asarray(inputs["sca_b"], np.float32)
    w3 = np.asarray(inputs["w3"], np.float32)
    b3 = np.asarray(inputs["b3"], np.float32)
    w4 = np.asarray(inputs["w4"], np.float32)
    b4 = np.asarray(inputs["b4"], np.float32)
    w5 = np.asarray(inputs["w5"], np.float32)
    b5 = np.asarray(inputs["b5"], np.float32)
    ln1_w = np.asarray(inputs["ln1_w"], np.float32).reshape(C)
    ln2_w = np.asarray(inputs["ln2_w"], np.float32).reshape(C)
    beta = np.asarray(inputs["beta"], np.float32).reshape(C)
    gamma = np.asarray(inputs["gamma"], np.float32).reshape(C)

    s = style @ style_w.T + style_b                     # [B, DW]
    k2 = (mod_w ** 2).sum(axis=(1, 2, 3, 4))            # [DW]
    demod = 1.0 / np.sqrt(k2[None] * s * s + 1e-8)      # [B, DW]
    sdv = s * demod                                     # [B, DW]

    W1t = w1 * ln1_w[None, :]                           # [DW, C]
    wdw = mod_w[:, 0]                                   # [DW, 3,3,3]

    def wtap(kind, kd, kh):
        m = np.zeros((128, 128), np.float32)
        if kind == 0:   # P: kw0 lower, kw1 upper (upper band = x shifted +1)
            m[0:64] = (W1t * wdw[:, kd, kh, 0][:, None]).T
            m[64:128] = (W1t * wdw[:, kd, kh, 1][:, None]).T
        else:           # S: kw2 lower only
            m[0:64] = (W1t * wdw[:, kd, kh, 2][:, None]).T
        return m

    wPS = np.zeros((128, 9, 2, 128), np.float32)
    for q, (t1, t2) in enumerate(DRPAIRS):
        wPS[:, q, 0] = wtap(*t1)
        wPS[:, q, 1] = wtap(*t2)

    # boundary-correction coefficients (b1 * sum of valid taps)
    def S(cd, ch, cw):
        vd = {0: [1, 2], 1: [0, 1, 2], 2: [0, 1]}[cd]
        vh = {0: [1, 2], 1: [0, 1, 2], 2: [0, 1]}[ch]
        vw = {0: [1, 2], 1: [0, 1, 2], 2: [0, 1]}[cw]
        return wdw[:, vd][:, :, vh][:, :, :, vw].sum(axis=(1, 2, 3))

    g = np.zeros((9, 64, 64), np.float32)
    g[0] = 1.0
    g[1, 0, :] = 1.0
    g[2, 63, :] = 1.0
    g[3, :, 0] = 1.0
    g[4, :, 63] = 1.0
    g[5, 0, 0] = 1.0
    g[6, 0, 63] = 1.0
    g[7, 63, 0] = 1.0
    g[8, 63, 63] = 1.0
    ind = np.zeros((9, 3, 512), np.float32)
    ind[:, 0] = g[:, 0:8, :].reshape(9, -1)
    ind[:, 1] = g[:, 8:16, :].reshape(9, -1)
    ind[:, 2] = g[:, 56:64, :].reshape(9, -1)

    def corr_for(dcase):
        c = np.zeros((9, 128), np.float32)
        base = S(dcase, 1, 1)
        c[0] = base
        c[1] = S(dcase, 0, 1) - base
        c[2] = S(dcase, 2, 1) - base
        c[3] = S(dcase, 1, 0) - base
        c[4] = S(dcase, 1, 2) - base
        c[5] = S(dcase, 0, 0) - S(dcase, 0, 1) - S(dcase, 1, 0) + base
        c[6] = S(dcase, 0, 2) - S(dcase, 0, 1) - S(dcase, 1, 2) + base
        c[7] = S(dcase, 2, 0) - S(dcase, 2, 1) - S(dcase, 1, 0) + base
        c[8] = S(dcase, 2, 2) - S(dcase, 2, 1) - S(dcase, 1, 2) + base
        return c * b1[None, :]

    corr_tab = {c: corr_for(c) for c in (0, 1, 2)}

    blob8 = np.zeros((128, 8064), f8)
    blob8[:, 0:2304] = (wPS * S_W).astype(f8).reshape(128, 2304)
    blob8[:, 2304:2432] = np.eye(128, dtype=np.float32).astype(f8)
    blob8[0:9, 6528:8064] = ind.astype(f8).reshape(9, 1536)
    blobb = np.zeros((128, 512), bf)
    blobb[:, 0:64] = (w3.T * beta[None, :]).astype(bf)
    blobb[:, 64:192] = (sca_w.T / float(D * H * W)).astype(bf)
    blobb[0:64, 192:320] = (w4 * ln2_w[None, :]).T.astype(bf)
    blobb[:, 320:384] = (w5.T * gamma[None, :]).astype(bf)
    blobb[:, 384:512] = np.eye(128, dtype=np.float32).astype(bf)
    blobf = np.zeros((128, 70), np.float32)
    blobf[:, 1] = mod_b
    blobf[:, 2] = sca_b
    blobf[0:64, 3] = b3 * beta
    blobf[:, 4] = b4
    blobf[0:64, 5] = b5 * gamma
    blobf[0:64, 6] = b3 * beta + b5 * gamma
    common = dict(blob8=blob8, blobb=blobb,
                  i64f=np.eye(64, dtype=np.float32))

    in_maps = []
    for k in range(8):
        b, d0 = k // 4, (k % 4) * NPL
        ip = inp[b]
        halo = np.zeros((NHALO, C, HWC), np.float32)
        lo, hi = max(d0 - 1, 0), min(d0 + NPL + 1, D)
        halo[lo - (d0 - 1):hi - (d0 - 1)] = (
            ip[:, lo:hi].transpose(1, 0, 2, 3).reshape(hi - lo, C, HWC))
        wcorr = np.zeros((9, NPL, 2, 128), np.float32)
        for i in range(NPL):
            dg = d0 + i
            dcase = 0 if dg == 0 else (2 if dg == D - 1 else 1)
            wcorr[:, i, 0, :] = corr_tab[dcase]
        m = dict(common)
        m["inp_t"] = halo.astype(bf)
        m["inp_f"] = np.ascontiguousarray(
            ip[:, d0:d0 + NPL].transpose(1, 0, 2, 3).reshape(NPL, C, HWC))
        b8 = blob8.copy()
        b8[0:9, 2432:6528] = (wcorr * S_W).astype(f8).reshape(9, 4096)
        m["blob8"] = b8
        bff = blobf.copy()
        bff[:, 0] = (sdv[b] / S_W)
        m["blobf"] = bff
        in_maps.append(m)
    return in_maps


def kernel(**inputs):
    from concourse.bass_utils import run_bass_kernel_spmd
    if "nc" not in _CACHE:
        _CACHE["nc"] = _build()
    nc = _CACHE["nc"]
    in_maps = _host_prep(inputs)
    res = run_bass_kernel_spmd(nc, in_maps, list(range(8)))
    _CACHE["last_res"] = res
    out = np.empty((2, C, D, H, W), np.float32)
    for k in range(8):
        b, d0 = k // 4, (k % 4) * NPL
        o = res.results[k]["out"]
        out[b, :, d0:d0 + NPL] = o.reshape(NPL, C, H, W).transpose(1, 0, 2, 3)
    return out


# revision 4
# speedup vs baseline: 1.0150x; 1.0150x over previous
"""Trainium2 Bass kernel for nn_BaselineBlock_SCA_Modulated — v2.

Sharding: 8 cores = 2 batch x 4 D-slabs of 16 planes. Halo planes staged
host-side (zeros at global D edges) so all cores run one SPMD program.

v2 changes vs baseline:
- Conv chain in fp8 (e4m3) with DoubleRow matmuls: 27 taps + boundary corr
  in 11 matmuls / 7 N-units per 512-chunk (vs 16 bf16 matmuls).
- Gelu output (xg) stays resident in SBUF as fp8 (no DRAM round trip).
- LN stats via bn_stats; dual-shift ring copies via uint16-bitcast 2x copies.
- Residual adds done on PE (identity f32r matmuls into PSUM) with Act
  readout, instead of DVE affine ops.
- DMA queues split: loads on SP, stores on Pool.
"""
import numpy as np
import ml_dtypes

C, DW, SD = 64, 128, 512
D, H, W = 64, 64, 64
NPL = 16              # output planes per core
NHALO = NPL + 2       # input planes incl halo
PW = 68               # padded row width (2 left pad + 64 + 2 right pad)
NPR = 66              # padded row count (1 top + 64 + 1 bottom)
PSZ = NPR * PW        # padded plane size (even)
HWC = H * W           # 4096
NCH = 32              # 128-position chunks per plane
EPS = 1e-6
S_W = 256.0           # fp8 weight prescale
bf = ml_dtypes.bfloat16
f8 = ml_dtypes.float8_e4m3fn

# (kind, kd, kh): kind 0 = P-read (kw0 lower / kw1 upper, col 1),
# kind 1 = S-read (kw2 lower, zero upper, col 3)
TAPS = [(0, kd, kh) for kd in range(3) for kh in range(3)] + \
       [(1, kd, kh) for kd in range(3) for kh in range(3)]
DRPAIRS = [(TAPS[2 * i], TAPS[2 * i + 1]) for i in range(9)]

_CACHE = {}


def _build():
    import concourse.bacc as bacc
    import concourse.mybir as mybir
    import concourse.tile as tile
    from concourse.ap import AP
    from concourse.mybir import ActivationFunctionType as AF, AluOpType as ALU

    BF = mybir.dt.bfloat16
    F32 = mybir.dt.float32
    F32R = mybir.dt.float32r
    FP8 = mybir.dt.float8e4
    U16 = mybir.dt.uint16
    AX = mybir.AxisListType
    DR = mybir.MatmulPerfMode.DoubleRow

    nc = bacc.Bacc("TRN2", target_bir_lowering=False, debug=False, num_devices=8)

    dram = {}
    def din(name, shape, dt=BF):
        dram[name] = nc.dram_tensor(name, shape, dt, kind="ExternalInput")
        return dram[name]

    inp_t = din("inp_t", [NHALO, C, HWC], BF)
    inp_f = din("inp_f", [NPL, C, HWC], F32R)
    blob8_i = din("blob8", [128, 8064], FP8)
    blobb_i = din("blobb", [128, 512], BF)
    blobf_i = din("blobf", [128, 70], F32)
    i64f_i = din("i64f", [64, 64], F32R)
    out_d = nc.dram_tensor("out", [NPL, C, HWC], F32, kind="ExternalOutput")

    cc_a = nc.dram_tensor("cc_a", [128, 1], F32)
    cc_b = nc.dram_tensor("cc_b", [128, 1], F32)

    from contextlib import ExitStack
    with tile.TileContext(nc) as tc, ExitStack() as stk:
        cpool = stk.enter_context(tc.tile_pool(name="const", bufs=1))
        sm = stk.enter_context(tc.tile_pool(name="small", bufs=3))
        p1stk = ExitStack()
        wp1 = p1stk.enter_context(tc.tile_pool(name="p1", bufs=2))
        rpool = p1stk.enter_context(tc.tile_pool(name="ring", bufs=1))
        psC = p1stk.enter_context(tc.tile_pool(name="psC", bufs=4,
                                               space="PSUM"))

        def const(name, shape, dt):
            t = cpool.tile(shape, dt, tag=name, name=name)
            nc.sync.dma_start(t[:], dram[name][:])
            return t

        t8 = const("blob8", [128, 8064], FP8)
        tb = const("blobb", [128, 512], BF)
        tf = const("blobf", [128, 70], F32)
        i64t = const("i64f", [64, 64], F32R)
        wPS = t8[:, 0:2304].rearrange("p (q t m) -> p q t m", q=9, t=2)
        i128f8 = t8[:, 2304:2432]
        wc = t8[0:9, 2432:6528].rearrange("p (d t m) -> p d t m", d=16, t=2)
        ind = t8[0:9, 6528:8064].rearrange("p (a b) -> p a b", a=3)
        w3T = tb[:, 0:64]
        scawT = tb[:, 64:192]
        w4T = tb[0:64, 192:320]
        w5T = tb[:, 320:384]
        i128 = tb[:, 384:512]
        sd = tf[:, 0:1]
        modb = tf[:, 1:2]
        scab = tf[:, 2:3]
        b3beta = tf[0:64, 3:4]
        b4 = tf[:, 4:5]
        b5g = tf[0:64, 5:6]
        b35 = tf[0:64, 6:7]
        i64f = i64t[:]

        pools = cpool.tile([128, NPL * 8], F32, tag="pools")
        w3Tp = cpool.tile([128, 64], BF, tag="w3Tp")
        # persistent ring (4 slots in ONE tensor for cross-slot DoubleRow
        # k-tile reads). fp8 values live at byte stride 2 (the fp8 PE
        # transpose writes with element step 2), so the ring is a U16 tile
        # whose low... each u16 cell holds one fp8 value; u16 copies move it.
        ring = rpool.tile([128, 4 * PSZ], U16, tag="ring", name="ring")
        xga = cpool.tile([128, NPL * HWC], FP8, tag="xga", name="xga")
        nc.gpsimd.memset(ring[:], 0)

        rAP = ring[:]
        ring_pstride = rAP.ap[0][0]
        r8 = rAP.bitcast(FP8)

        def rd(parts, offv, dims):
            """fp8 AP into the ring; offv/dims in VALUE units (1 value =
            2 bytes = 2 fp8 elements; strides passed here get doubled)."""
            return AP(r8.tensor, r8.offset + 2 * offv,
                      [[r8.ap[0][0], parts]] + [[2 * s, n] for s, n in dims])

        # ---------------- PASS 1 ----------------
        def ln_stats(xT, sfx, pool):
            """per-(partition,chunk) LN stats over the 64-ch innermost dim.
            Returns (rv, mrv) [128, NCH] f32."""
            sq = pool.tile([128, NCH, 64], BF, tag="sq" + sfx, bufs=1)
            nc.vector.tensor_mul(sq[:], xT[:], xT[:])
            msum = sm.tile([128, NCH], BF, tag="msum" + sfx)
            qsum = sm.tile([128, NCH], BF, tag="qsum" + sfx)
            with nc.allow_low_precision(reason="stat sums; dve accumulates "
                                        "fp32 internally, bf16 round-off is "
                                        "far below the fp8 conv noise"):
                nc.vector.tensor_reduce(msum[:], xT[:], axis=AX.X, op=ALU.add)
                nc.vector.tensor_reduce(qsum[:], sq[:], axis=AX.X, op=ALU.add)
            t1v = sm.tile([128, NCH], F32, tag="t1v" + sfx)
            nc.vector.tensor_mul(t1v[:], msum[:], msum[:])
            t3v = sm.tile([128, NCH], F32, tag="t3v" + sfx)
            nc.vector.tensor_scalar_mul(t3v[:], qsum[:], 1.0 / 63.0)
            var = sm.tile([128, NCH], F32, tag="var" + sfx)
            nc.vector.scalar_tensor_tensor(
                var[:], t1v[:], -1.0 / (64.0 * 63.0), t3v[:],
                op0=ALU.mult, op1=ALU.add)
            # rsqrt(var) via Newton with r0 = 2/(1+v): a global underestimate
            # of rsqrt (AM-GM), so iteration converges monotonically; keeps
            # ACT parked on the gelu table set. eps (1e-6 on std ~1) is far
            # below fp8/bf16 noise and is dropped.
            sv = sm.tile([128, NCH], F32, tag="sv" + sfx)
            nc.vector.tensor_scalar(sv[:], var[:], 0.5, 0.5,
                                    op0=ALU.mult, op1=ALU.add)
            rv = sm.tile([128, NCH], F32, tag="rv" + sfx)
            nc.vector.reciprocal(rv[:], sv[:])
            tq = sm.tile([128, NCH], F32, tag="tq" + sfx)
            for _ in range(2):
                nc.vector.tensor_mul(tq[:], rv[:], rv[:])
                nc.vector.tensor_mul(tq[:], tq[:], var[:])
                nc.vector.tensor_scalar(tq[:], tq[:], -0.5, 1.5,
                                        op0=ALU.mult, op1=ALU.add)
                nc.vector.tensor_mul(rv[:], rv[:], tq[:])
            mrv = sm.tile([128, NCH], F32, tag="mrv" + sfx)
            nc.vector.scalar_tensor_tensor(
                mrv[:], msum[:], 1.0 / 64.0, rv[:], op0=ALU.mult, op1=ALU.mult)
            return rv, mrv

        xl8s = {}

        def ln1a(p):
            xT = wp1.tile([128, NCH, 64], BF, tag="xT", bufs=3)
            nc.sync.dma_start_transpose(xT[:], inp_t[p])
            rv, mrv = ln_stats(xT, "1", wp1)
            # apply LN (mul in place, sub quantizes to fp8)
            rvv = rv[:].unsqueeze(2).broadcast_to([128, NCH, 64])
            mrvv = mrv[:].unsqueeze(2).broadcast_to([128, NCH, 64])
            nc.vector.tensor_mul(xT[:], xT[:], rvv)
            xl8 = wp1.tile([128, NCH, 64], FP8, tag="xl8", bufs=3)
            nc.vector.tensor_sub(xl8[:], xT[:], mrvv)
            xl8s[p] = xl8

        def ln1b(p):
            slot = p % 4
            xl8 = xl8s.pop(p)
            # transpose to ring layout: fp8 transposes write with element
            # step 2 (hw requirement) into psF; u16 copies move value cells.
            for hf in range(2):
                psF = psC.tile([64, HWC], FP8, tag="trF", bufs=2)
                for g in range(16):
                    dst = psF[:, g * 256:(g + 1) * 256].rearrange(
                        "p (c two) -> p c two", two=2)[:, :, 0]
                    nc.tensor.transpose(dst, xl8[:, 16 * hf + g, :], i128f8)
                srcu = psF[:].bitcast(U16).rearrange(
                    "p (r w) -> p r w", w=64)
                o_lo = slot * PSZ + (1 + 32 * hf) * PW + 2
                dst_lo = AP(rAP.tensor, rAP.offset + o_lo,
                            [[ring_pstride, 64], [PW, 32], [1, 64]])
                nc.scalar.copy(dst_lo, srcu)
            # upper dual band (x shifted +1 value) via one contiguous
            # Pool-issued DMA over the full interior row block
            src_full = AP(rAP.tensor, rAP.offset + slot * PSZ + PW,
                          [[ring_pstride, 64], [1, 64 * PW]])
            dst_full = AP(rAP.tensor,
                          rAP.offset + 64 * ring_pstride + slot * PSZ + PW - 1,
                          [[ring_pstride, 64], [1, 64 * PW]])
            nc.gpsimd.dma_start(dst_full, src_full)

        def aoff(slots, kd, kh, cb):
            return slots[kd] * PSZ + (8 * cb + kh) * PW

        def conv_plane(d):
            slots = [(d + kd) % 4 for kd in range(3)]
            pat = lambda cb: 0 if cb == 0 else (2 if cb == 7 else 1)
            for cb in range(8):
                ps = psC.tile([128, 512], F32, tag="mm", bufs=4)
                first = True
                for q, (t1, t2) in enumerate(DRPAIRS):
                    o1 = aoff(slots, t1[1], t1[2], cb) + 1 + 2 * t1[0]
                    o2 = aoff(slots, t2[1], t2[2], cb) + 1 + 2 * t2[0]
                    rhs = rd(128, o1, [[o2 - o1, 2], [PW, 8], [1, 64]])
                    nc.tensor.matmul(ps[:], wPS[:, q], rhs, start=first,
                                     stop=False, perf_mode=DR,
                                     skip_group_check=True)
                    first = False
                rhs_c = ind[:, pat(cb), :].unsqueeze(1).broadcast_to(
                    [9, 2, 512])
                nc.tensor.matmul(ps[:], wc[:, d], rhs_c, start=False,
                                 stop=True, perf_mode=DR,
                                 skip_group_check=True)
                nc.scalar.activation(
                    xga[:, d * HWC + cb * 512: d * HWC + (cb + 1) * 512],
                    ps[:], AF.Gelu, bias=modb, scale=sd,
                    accum_out=pools[:, d * 8 + cb:d * 8 + cb + 1])

        for r in range(NHALO + 2):
            if r < NHALO:
                ln1a(r)
            if r >= 1 and r - 1 < NHALO:
                ln1b(r - 1)
            if r >= 4:
                conv_plane(r - 4)

        p1stk.close()
        p2 = stk.enter_context(tc.tile_pool(name="p2", bufs=2))
        psB2 = stk.enter_context(tc.tile_pool(name="psB2", bufs=2,
                                              space="PSUM"))

        # ---------------- pooled -> gate ----------------
        pooled = cpool.tile([128, 1], F32, tag="pooled")
        nc.vector.tensor_reduce(pooled[:], pools[:], axis=AX.X, op=ALU.add)
        nc.gpsimd.dma_start(cc_a[:], pooled[:])
        nc.gpsimd.collective_compute(
            "AllReduce", ALU.add,
            replica_groups=[[0, 1, 2, 3], [4, 5, 6, 7]],
            ins=[cc_a[:]], outs=[cc_b[:]])
        pooled2f = cpool.tile([128, 1], F32, tag="pooled2f", name="pooled2f")
        nc.gpsimd.dma_start(pooled2f[:], cc_b[:])
        pooled2 = cpool.tile([128, 1], BF, tag="pooled2", name="pooled2")
        nc.vector.tensor_copy(pooled2[:], pooled2f[:])
        psg = psB2.tile([128, 512], F32, tag="m45", bufs=2)
        nc.tensor.matmul(psg[:, 0:1], scawT, pooled2[:], start=True,
                         stop=True)
        gate = cpool.tile([128, 1], F32, tag="gatev")
        nc.scalar.activation(gate[:], psg[:, 0:1], AF.Identity, bias=scab)
        nc.vector.tensor_scalar_mul(w3Tp[:], w3T, gate[:])

        # ---------------- PASS 2 ----------------
        ys = {}
        xl2s = {}

        def p2a1(d):
            xg_d = xga[:, d * HWC:(d + 1) * HWC]
            y = p2.tile([64, HWC], F32R, tag="y", bufs=2)
            ys[d] = y
            for hf in range(2):
                ifp = p2.tile([64, 2048], F32R, tag="ifp", bufs=2)
                nc.sync.dma_start(ifp[:], inp_f[d][:, hf * 2048:(hf + 1) * 2048])
                for cq in range(4):
                    cb = 4 * hf + cq
                    sl = slice(cb * 512, (cb + 1) * 512)
                    ps3 = psB2.tile([64, 512], F32, tag="m3", bufs=3)
                    nc.tensor.matmul(ps3[:], w3Tp[:], xg_d[:, sl],
                                     start=True, stop=False,
                                     skip_group_check=True)
                    nc.tensor.matmul(
                        ps3[:], i64f, ifp[:, cq * 512:(cq + 1) * 512],
                        start=False, stop=True, skip_group_check=True)
                    nc.scalar.activation(y[:, sl], ps3[:], AF.Identity,
                                         bias=b3beta)

        def p2a2(d):
            y = ys.pop(d)
            # LN2 stats (transposed layout)
            yTs = p2.tile([128, NCH, 64], BF, tag="yTs")
            for qf in range(4):
                psT = psB2.tile([128, 512], F32R, tag="tr")
                for g in range(8):
                    cg = 8 * qf + g
                    nc.tensor.transpose(psT[:, g * 64:(g + 1) * 64],
                                        y[:, cg * 128:(cg + 1) * 128], i64f)
                nc.vector.tensor_copy(
                    yTs[:, 8 * qf:8 * (qf + 1), :],
                    psT[:].rearrange("p (g c) -> p g c", c=64))
            rv, mrv = ln_stats(yTs, "2", p2)
            rvv = rv[:].unsqueeze(2).broadcast_to([128, NCH, 64])
            mrvv = mrv[:].unsqueeze(2).broadcast_to([128, NCH, 64])
            nc.vector.tensor_mul(yTs[:], yTs[:], rvv)
            xl2 = p2.tile([128, NCH, 64], BF, tag="xl2")
            nc.vector.tensor_sub(xl2[:], yTs[:], mrvv)
            xl2s[d] = xl2

        xln2s = {}

        def p2b1(d):
            xl2 = xl2s.pop(d)
            xln2 = p2.tile([64, HWC], BF, tag="xln2")
            for hf in range(2):
                for cq in range(4):
                    psX = psB2.tile([64, 512], BF, tag="x4", bufs=1)
                    for g in range(4):
                        cg = 16 * hf + 4 * cq + g
                        nc.tensor.transpose(psX[:, g * 128:(g + 1) * 128],
                                            xl2[:, cg, :], i128)
                    nc.vector.tensor_copy(
                        xln2[:, hf * 2048 + cq * 512:
                             hf * 2048 + (cq + 1) * 512], psX[:])
            xln2s[d] = xln2

        def p2b2(d):
            # out = inp + beta*pw3 + b3beta + gamma*pw5 + b5gamma: the pw3
            # and inp terms are re-accumulated here (instead of adding the
            # staged y) so y only has to live two pipeline rounds.
            xg_d = xga[:, d * HWC:(d + 1) * HWC]
            xln2 = xln2s.pop(d)
            for hf in range(2):
                if2 = p2.tile([64, 2048], F32R, tag="if2", bufs=2)
                nc.gpsimd.dma_start(if2[:],
                                     inp_f[d][:, hf * 2048:(hf + 1) * 2048])
                outp = p2.tile([64, 2048], F32, tag="outp", bufs=2)
                xg2 = p2.tile([128, 2048], BF, tag="xg2")
                for cq in range(4):
                    cb = 4 * hf + cq
                    sl = slice(cb * 512, (cb + 1) * 512)
                    lsl = slice(cq * 512, (cq + 1) * 512)
                    ps4 = psB2.tile([128, 512], F32, tag="m45", bufs=2)
                    nc.tensor.matmul(ps4[:], w4T, xln2[:, sl],
                                     start=True, stop=True)
                    nc.scalar.activation(xg2[:, lsl], ps4[:], AF.Gelu,
                                         bias=b4)
                    ps5 = psB2.tile([64, 512], F32, tag="m45", bufs=2)
                    nc.tensor.matmul(ps5[:], w5T, xg2[:, lsl],
                                     start=True, stop=False,
                                     skip_group_check=True)
                    nc.tensor.matmul(ps5[:], w3Tp[:], xg_d[:, sl],
                                     start=False, stop=False,
                                     skip_group_check=True)
                    nc.tensor.matmul(ps5[:], i64f,
                                     if2[:, cq * 512:(cq + 1) * 512],
                                     start=False, stop=True,
                                     skip_group_check=True)
                    nc.scalar.activation(outp[:, lsl], ps5[:], AF.Identity,
                                         bias=b35)
                nc.gpsimd.dma_start(
                    out_d[d][:, hf * 2048:(hf + 1) * 2048], outp[:])

        for r in range(NPL + 3):
            if r < NPL:
                p2a1(r)
            if r >= 1 and r - 1 < NPL:
                p2a2(r - 1)
            if r >= 2 and r - 2 < NPL:
                p2b1(r - 2)
            if r >= 3:
                p2b2(r - 3)

    nc.compile()
    return nc


def _host_prep(inputs):
    inp = np.asarray(inputs["inp"], np.float32)
    style = np.asarray(inputs["style_vector"], np.float32)
    w1 = np.asarray(inputs["w1"], np.float32)
    b1 = np.asarray(inputs["b1"], np.float32)
    mod_w = np.asarray(inputs["mod_w"], np.float32)
    mod_b = np.asarray(inputs["mod_b"], np.float32)
    style_w = np.asarray(inputs["style_w"], np.float32)
    style_b = np.asarray(inputs["style_b"], np.float32)
    sca_w = np.asarray(inputs["sca_w"], np.float32)
    sca_b = np.asarray(inputs["sca_b"], np.float32)
    w3 = np.asarray(inputs["w3"], np.float32)
    b3 = np.asarray(inputs["b3"], np.float32)
    w4 = np.asarray(inputs["w4"], np.float32)
    b4 = np.asarray(inputs["b4"], np.float32)
    w5 = np.asarray(inputs["w5"], np.float32)
    b5 = np.asarray(inputs["b5"], np.float32)
    ln1_w = np.asarray(inputs["ln1_w"], np.float32).reshape(C)
    ln2_w = np.asarray(inputs["ln2_w"], np.float32).reshape(C)
    beta = np.asarray(inputs["beta"], np.float32).reshape(C)
    gamma = np.asarray(inputs["gamma"], np.float32).reshape(C)

    s = style @ style_w.T + style_b                     # [B, DW]
    k2 = (mod_w ** 2).sum(axis=(1, 2, 3, 4))            # [DW]
    demod = 1.0 / np.sqrt(k2[None] * s * s + 1e-8)      # [B, DW]
    sdv = s * demod                                     # [B, DW]

    W1t = w1 * ln1_w[None, :]                           # [DW, C]
    wdw = mod_w[:, 0]                                   # [DW, 3,3,3]

    def wtap(kind, kd, kh):
        m = np.zeros((128, 128), np.float32)
        if kind == 0:   # P: kw0 lower, kw1 upper (upper band = x shifted +1)
            m[0:64] = (W1t * wdw[:, kd, kh, 0][:, None]).T
            m[64:128] = (W1t * wdw[:, kd, kh, 1][:, None]).T
        else:           # S: kw2 lower only
            m[0:64] = (W1t * wdw[:, kd, kh, 2][:, None]).T
        return m

    wPS = np.zeros((128, 9, 2, 128), np.float32)
    for q, (t1, t2) in enumerate(DRPAIRS):
        wPS[:, q, 0] = wtap(*t1)
        wPS[:, q, 1] = wtap(*t2)

    # boundary-correction coefficients (b1 * sum of valid taps)
    def S(cd, ch, cw):
        vd = {0: [1, 2], 1: [0, 1, 2], 2: [0, 1]}[cd]
        vh = {0: [1, 2], 1: [0, 1, 2], 2: [0, 1]}[ch]
        vw = {0: [1, 2], 1: [0, 1, 2], 2: [0, 1]}[cw]
        return wdw[:, vd][:, :, vh][:, :, :, vw].sum(axis=(1, 2, 3))

    g = np.zeros((9, 64, 64), np.float32)
    g[0] = 1.0
    g[1, 0, :] = 1.0
    g[2, 63, :] = 1.0
    g[3, :, 0] = 1.0
    g[4, :, 63] = 1.0
    g[5, 0, 0] = 1.0
    g[6, 0, 63] = 1.0
    g[7, 63, 0] = 1.0
    g[8, 63, 63] = 1.0
    ind = np.zeros((9, 3, 512), np.float32)
    ind[:, 0] = g[:, 0:8, :].reshape(9, -1)
    ind[:, 1] = g[:, 8:16, :].reshape(9, -1)
    ind[:, 2] = g[:, 56:64, :].reshape(9, -1)

    def corr_for(dcase):
        c = np.zeros((9, 128), np.float32)
        base = S(dcase, 1, 1)
        c[0] = base
        c[1] = S(dcase, 0, 1) - base
        c[2] = S(dcase, 2, 1) - base
        c[3] = S(dcase, 1, 0) - base
        c[4] = S(dcase, 1, 2) - base
        c[5] = S(dcase, 0, 0) - S(dcase, 0, 1) - S(dcase, 1, 0) + base
        c[6] = S(dcase, 0, 2) - S(dcase, 0, 1) - S(dcase, 1, 2) + base
        c[7] = S(dcase, 2, 0) - S(dcase, 2, 1) - S(dcase, 1, 0) + base
        c[8] = S(dcase, 2, 2) - S(dcase, 2, 1) - S(dcase, 1, 2) + base
        return c * b1[None, :]

    corr_tab = {c: corr_for(c) for c in (0, 1, 2)}

    blob8 = np.zeros((128, 8064), f8)
    blob8[:, 0:2304] = (wPS * S_W).astype(f8).reshape(128, 2304)
    blob8[:, 2304:2432] = np.eye(128, dtype=np.float32).astype(f8)
    blob8[0:9, 6528:8064] = ind.astype(f8).reshape(9, 1536)
    blobb = np.zeros((128, 512), bf)
    blobb[:, 0:64] = (w3.T * beta[None, :]).astype(bf)
    blobb[:, 64:192] = (sca_w.T / float(D * H * W)).astype(bf)
    blobb[0:64, 192:320] = (w4 * ln2_w[None, :]).T.astype(bf)
    blobb[:, 320:384] = (w5.T * gamma[None, :]).astype(bf)
    blobb[:, 384:512] = np.eye(128, dtype=np.float32).astype(bf)
    blobf = np.zeros((128, 70), np.float32)
    blobf[:, 1] = mod_b
    blobf[:, 2] = sca_b
    blobf[0:64, 3] = b3 * beta
    blobf[:, 4] = b4
    blobf[0:64, 5] = b5 * gamma
    blobf[0:64, 6] = b3 * beta + b5 * gamma
    common = dict(blob8=blob8, blobb=blobb,
                  i64f=np.eye(64, dtype=np.float32))

    in_maps = []
    for k in range(8):
        b, d0 = k // 4, (k % 4) * NPL
        ip = inp[b]
        halo = np.zeros((NHALO, C, HWC), np.float32)
        lo, hi = max(d0 - 1, 0), min(d0 + NPL + 1, D)
        halo[lo - (d0 - 1):hi - (d0 - 1)] = (
            ip[:, lo:hi].transpose(1, 0, 2, 3).reshape(hi - lo, C, HWC))
        wcorr = np.zeros((9, NPL, 2, 128), np.float32)
        for i in range(NPL):
            dg = d0 + i
            dcase = 0 if dg == 0 else (2 if dg == D - 1 else 1)
            wcorr[:, i, 0, :] = corr_tab[dcase]
        m = dict(common)
        m["inp_t"] = halo.astype(bf)
        m["inp_f"] = np.ascontiguousarray(
            ip[:, d0:d0 + NPL].transpose(1, 0, 2, 3).reshape(NPL, C, HWC))
        b8 = blob8.copy()
        b8[0:9, 2432:6528] = (wcorr * S_W).astype(f8).reshape(9, 4096)
        m["blob8"] = b8
        bff = blobf.copy()
        bff[:, 0] = (sdv[b] / S_W)
        m["blobf"] = bff
        in_maps.append(m)
    return in_maps


def kernel(**inputs):
    from concourse.bass_utils import run_bass_kernel_spmd
    if "nc" not in _CACHE:
        _CACHE["nc"] = _build()
    nc = _CACHE["nc"]
    in_maps = _host_prep(inputs)
    res = run_bass_kernel_spmd(nc, in_maps, list(range(8)))
    _CACHE["last_res"] = res
    out = np.empty((2, C, D, H, W), np.float32)
    for k in range(8):
        b, d0 = k // 4, (k % 4) * NPL
        o = res.results[k]["out"]
        out[b, :, d0:d0 + NPL] = o.reshape(NPL, C, H, W).transpose(1, 0, 2, 3)
    return out


# revision 5
# speedup vs baseline: 1.0169x; 1.0019x over previous
"""Trainium2 Bass kernel for nn_BaselineBlock_SCA_Modulated — v2.

Sharding: 8 cores = 2 batch x 4 D-slabs of 16 planes. Halo planes staged
host-side (zeros at global D edges) so all cores run one SPMD program.

v2 changes vs baseline:
- Conv chain in fp8 (e4m3) with DoubleRow matmuls: 27 taps + boundary corr
  in 11 matmuls / 7 N-units per 512-chunk (vs 16 bf16 matmuls).
- Gelu output (xg) stays resident in SBUF as fp8 (no DRAM round trip).
- LN stats via bn_stats; dual-shift ring copies via uint16-bitcast 2x copies.
- Residual adds done on PE (identity f32r matmuls into PSUM) with Act
  readout, instead of DVE affine ops.
- DMA queues split: loads on SP, stores on Pool.
"""
import numpy as np
import ml_dtypes

C, DW, SD = 64, 128, 512
D, H, W = 64, 64, 64
NPL = 16              # output planes per core
NHALO = NPL + 2       # input planes incl halo
PW = 68               # padded row width (2 left pad + 64 + 2 right pad)
NPR = 66              # padded row count (1 top + 64 + 1 bottom)
PSZ = NPR * PW        # padded plane size (even)
HWC = H * W           # 4096
NCH = 32              # 128-position chunks per plane
EPS = 1e-6
S_W = 256.0           # fp8 weight prescale
bf = ml_dtypes.bfloat16
f8 = ml_dtypes.float8_e4m3fn

# (kind, kd, kh): kind 0 = P-read (kw0 lower / kw1 upper, col 1),
# kind 1 = S-read (kw2 lower, zero upper, col 3)
TAPS = [(0, kd, kh) for kd in range(3) for kh in range(3)] + \
       [(1, kd, kh) for kd in range(3) for kh in range(3)]
DRPAIRS = [(TAPS[2 * i], TAPS[2 * i + 1]) for i in range(9)]

_CACHE = {}


def _build():
    import concourse.bacc as bacc
    import concourse.mybir as mybir
    import concourse.tile as tile
    from concourse.ap import AP
    from concourse.mybir import ActivationFunctionType as AF, AluOpType as ALU

    BF = mybir.dt.bfloat16
    F32 = mybir.dt.float32
    F32R = mybir.dt.float32r
    FP8 = mybir.dt.float8e4
    U16 = mybir.dt.uint16
    AX = mybir.AxisListType
    DR = mybir.MatmulPerfMode.DoubleRow

    nc = bacc.Bacc("TRN2", target_bir_lowering=False, debug=False, num_devices=8)

    dram = {}
    def din(name, shape, dt=BF):
        dram[name] = nc.dram_tensor(name, shape, dt, kind="ExternalInput")
        return dram[name]

    inp_t = din("inp_t", [NHALO, C, HWC], BF)
    inp_f = din("inp_f", [NPL, C, HWC], F32R)
    blob8_i = din("blob8", [128, 8064], FP8)
    blobb_i = din("blobb", [128, 512], BF)
    blobf_i = din("blobf", [128, 70], F32)
    i64f_i = din("i64f", [64, 64], F32R)
    out_d = nc.dram_tensor("out", [NPL, C, HWC], F32, kind="ExternalOutput")

    cc_a = nc.dram_tensor("cc_a", [128, 1], F32)
    cc_b = nc.dram_tensor("cc_b", [128, 1], F32)

    from contextlib import ExitStack
    with tile.TileContext(nc) as tc, ExitStack() as stk:
        cpool = stk.enter_context(tc.tile_pool(name="const", bufs=1))
        sm = stk.enter_context(tc.tile_pool(name="small", bufs=3))
        p1stk = ExitStack()
        wp1 = p1stk.enter_context(tc.tile_pool(name="p1", bufs=2))
        rpool = p1stk.enter_context(tc.tile_pool(name="ring", bufs=1))
        psC = p1stk.enter_context(tc.tile_pool(name="psC", bufs=4,
                                               space="PSUM"))

        def const(name, shape, dt):
            t = cpool.tile(shape, dt, tag=name, name=name)
            nc.sync.dma_start(t[:], dram[name][:])
            return t

        t8 = const("blob8", [128, 8064], FP8)
        tb = const("blobb", [128, 512], BF)
        tf = const("blobf", [128, 70], F32)
        i64t = const("i64f", [64, 64], F32R)
        wPS = t8[:, 0:2304].rearrange("p (q t m) -> p q t m", q=9, t=2)
        i128f8 = t8[:, 2304:2432]
        wc = t8[0:9, 2432:6528].rearrange("p (d t m) -> p d t m", d=16, t=2)
        ind = t8[0:9, 6528:8064].rearrange("p (a b) -> p a b", a=3)
        w3T = tb[:, 0:64]
        scawT = tb[:, 64:192]
        w4T = tb[0:64, 192:320]
        w5T = tb[:, 320:384]
        i128 = tb[:, 384:512]
        sd = tf[:, 0:1]
        modb = tf[:, 1:2]
        scab = tf[:, 2:3]
        b3beta = tf[0:64, 3:4]
        b4 = tf[:, 4:5]
        b5g = tf[0:64, 5:6]
        b35 = tf[0:64, 6:7]
        i64f = i64t[:]

        pools = cpool.tile([128, NPL * 8], F32, tag="pools")
        w3Tp = cpool.tile([128, 64], BF, tag="w3Tp")
        # persistent ring (4 slots in ONE tensor for cross-slot DoubleRow
        # k-tile reads). fp8 values live at byte stride 2 (the fp8 PE
        # transpose writes with element step 2), so the ring is a U16 tile
        # whose low... each u16 cell holds one fp8 value; u16 copies move it.
        ring = rpool.tile([128, 4 * PSZ], U16, tag="ring", name="ring")
        xga = cpool.tile([128, NPL * HWC], FP8, tag="xga", name="xga")
        nc.gpsimd.memset(ring[:], 0)

        rAP = ring[:]
        ring_pstride = rAP.ap[0][0]
        r8 = rAP.bitcast(FP8)

        def rd(parts, offv, dims):
            """fp8 AP into the ring; offv/dims in VALUE units (1 value =
            2 bytes = 2 fp8 elements; strides passed here get doubled)."""
            return AP(r8.tensor, r8.offset + 2 * offv,
                      [[r8.ap[0][0], parts]] + [[2 * s, n] for s, n in dims])

        # ---------------- PASS 1 ----------------
        def ln_stats(xT, sfx, pool):
            """per-(partition,chunk) LN stats over the 64-ch innermost dim.
            Returns (rv, mrv) [128, NCH] f32."""
            sq = pool.tile([128, NCH, 64], BF, tag="sq" + sfx, bufs=1)
            nc.vector.tensor_mul(sq[:], xT[:], xT[:])
            msum = sm.tile([128, NCH], BF, tag="msum" + sfx)
            qsum = sm.tile([128, NCH], BF, tag="qsum" + sfx)
            with nc.allow_low_precision(reason="stat sums; dve accumulates "
                                        "fp32 internally, bf16 round-off is "
                                        "far below the fp8 conv noise"):
                nc.vector.tensor_reduce(msum[:], xT[:], axis=AX.X, op=ALU.add)
                nc.vector.tensor_reduce(qsum[:], sq[:], axis=AX.X, op=ALU.add)
            t1v = sm.tile([128, NCH], F32, tag="t1v" + sfx)
            nc.vector.tensor_mul(t1v[:], msum[:], msum[:])
            t3v = sm.tile([128, NCH], F32, tag="t3v" + sfx)
            nc.vector.tensor_scalar_mul(t3v[:], qsum[:], 1.0 / 63.0)
            var = sm.tile([128, NCH], F32, tag="var" + sfx)
            nc.vector.scalar_tensor_tensor(
                var[:], t1v[:], -1.0 / (64.0 * 63.0), t3v[:],
                op0=ALU.mult, op1=ALU.add)
            # rsqrt(var) via Newton with r0 = 2/(1+v): a global underestimate
            # of rsqrt (AM-GM), so iteration converges monotonically; keeps
            # ACT parked on the gelu table set. eps (1e-6 on std ~1) is far
            # below fp8/bf16 noise and is dropped.
            sv = sm.tile([128, NCH], F32, tag="sv" + sfx)
            nc.vector.tensor_scalar(sv[:], var[:], 0.5, 0.5,
                                    op0=ALU.mult, op1=ALU.add)
            rv = sm.tile([128, NCH], F32, tag="rv" + sfx)
            nc.vector.reciprocal(rv[:], sv[:])
            tq = sm.tile([128, NCH], F32, tag="tq" + sfx)
            for _ in range(2):
                nc.vector.tensor_mul(tq[:], rv[:], rv[:])
                nc.vector.tensor_mul(tq[:], tq[:], var[:])
                nc.vector.tensor_scalar(tq[:], tq[:], -0.5, 1.5,
                                        op0=ALU.mult, op1=ALU.add)
                nc.vector.tensor_mul(rv[:], rv[:], tq[:])
            mrv = sm.tile([128, NCH], F32, tag="mrv" + sfx)
            nc.vector.scalar_tensor_tensor(
                mrv[:], msum[:], 1.0 / 64.0, rv[:], op0=ALU.mult, op1=ALU.mult)
            return rv, mrv

        xl8s = {}

        def ln1a(p):
            xT = wp1.tile([128, NCH, 64], BF, tag="xT", bufs=3)
            nc.sync.dma_start_transpose(xT[:], inp_t[p])
            rv, mrv = ln_stats(xT, "1", wp1)
            # apply LN (mul in place, sub quantizes to fp8)
            rvv = rv[:].unsqueeze(2).broadcast_to([128, NCH, 64])
            mrvv = mrv[:].unsqueeze(2).broadcast_to([128, NCH, 64])
            nc.vector.tensor_mul(xT[:], xT[:], rvv)
            xl8 = wp1.tile([128, NCH, 64], FP8, tag="xl8", bufs=3)
            nc.vector.tensor_sub(xl8[:], xT[:], mrvv)
            xl8s[p] = xl8

        def ln1b(p):
            slot = p % 4
            xl8 = xl8s.pop(p)
            # transpose to ring layout: fp8 transposes write with element
            # step 2 (hw requirement) into psF; u16 copies move value cells.
            for hf in range(2):
                psF = psC.tile([64, HWC], FP8, tag="trF", bufs=2)
                for g in range(16):
                    dst = psF[:, g * 256:(g + 1) * 256].rearrange(
                        "p (c two) -> p c two", two=2)[:, :, 0]
                    nc.tensor.transpose(dst, xl8[:, 16 * hf + g, :], i128f8)
                srcu = psF[:].bitcast(U16).rearrange(
                    "p (r w) -> p r w", w=64)
                o_lo = slot * PSZ + (1 + 32 * hf) * PW + 2
                dst_lo = AP(rAP.tensor, rAP.offset + o_lo,
                            [[ring_pstride, 64], [PW, 32], [1, 64]])
                nc.scalar.copy(dst_lo, srcu)
            # upper dual band (x shifted +1 value) via one contiguous
            # Pool-issued DMA over the full interior row block
            src_full = AP(rAP.tensor, rAP.offset + slot * PSZ + PW,
                          [[ring_pstride, 64], [1, 64 * PW]])
            dst_full = AP(rAP.tensor,
                          rAP.offset + 64 * ring_pstride + slot * PSZ + PW - 1,
                          [[ring_pstride, 64], [1, 64 * PW]])
            nc.gpsimd.dma_start(dst_full, src_full)

        def aoff(slots, kd, kh, cb):
            return slots[kd] * PSZ + (8 * cb + kh) * PW

        def conv_plane(d):
            slots = [(d + kd) % 4 for kd in range(3)]
            pat = lambda cb: 0 if cb == 0 else (2 if cb == 7 else 1)
            for cb in range(8):
                ps = psC.tile([128, 512], F32, tag="mm", bufs=4)
                first = True
                for q, (t1, t2) in enumerate(DRPAIRS):
                    o1 = aoff(slots, t1[1], t1[2], cb) + 1 + 2 * t1[0]
                    o2 = aoff(slots, t2[1], t2[2], cb) + 1 + 2 * t2[0]
                    rhs = rd(128, o1, [[o2 - o1, 2], [PW, 8], [1, 64]])
                    nc.tensor.matmul(ps[:], wPS[:, q], rhs, start=first,
                                     stop=False, perf_mode=DR,
                                     skip_group_check=True)
                    first = False
                rhs_c = ind[:, pat(cb), :].unsqueeze(1).broadcast_to(
                    [9, 2, 512])
                nc.tensor.matmul(ps[:], wc[:, d], rhs_c, start=False,
                                 stop=True, perf_mode=DR,
                                 skip_group_check=True)
                nc.scalar.activation(
                    xga[:, d * HWC + cb * 512: d * HWC + (cb + 1) * 512],
                    ps[:], AF.Gelu, bias=modb, scale=sd,
                    accum_out=pools[:, d * 8 + cb:d * 8 + cb + 1])

        for r in range(NHALO + 2):
            if r >= 4:
                conv_plane(r - 4)
            if r >= 1 and r - 1 < NHALO:
                ln1b(r - 1)
            if r < NHALO:
                ln1a(r)

        p1stk.close()
        p2 = stk.enter_context(tc.tile_pool(name="p2", bufs=2))
        psB2 = stk.enter_context(tc.tile_pool(name="psB2", bufs=2,
                                              space="PSUM"))

        # ---------------- pooled -> gate ----------------
        pooled = cpool.tile([128, 1], F32, tag="pooled")
        nc.vector.tensor_reduce(pooled[:], pools[:], axis=AX.X, op=ALU.add)
        nc.gpsimd.dma_start(cc_a[:], pooled[:])
        nc.gpsimd.collective_compute(
            "AllReduce", ALU.add,
            replica_groups=[[0, 1, 2, 3], [4, 5, 6, 7]],
            ins=[cc_a[:]], outs=[cc_b[:]])
        pooled2f = cpool.tile([128, 1], F32, tag="pooled2f", name="pooled2f")
        nc.gpsimd.dma_start(pooled2f[:], cc_b[:])
        pooled2 = cpool.tile([128, 1], BF, tag="pooled2", name="pooled2")
        nc.vector.tensor_copy(pooled2[:], pooled2f[:])
        psg = psB2.tile([128, 512], F32, tag="m45", bufs=2)
        nc.tensor.matmul(psg[:, 0:1], scawT, pooled2[:], start=True,
                         stop=True)
        gate = cpool.tile([128, 1], F32, tag="gatev")
        nc.scalar.activation(gate[:], psg[:, 0:1], AF.Identity, bias=scab)
        nc.vector.tensor_scalar_mul(w3Tp[:], w3T, gate[:])

        # ---------------- PASS 2 ----------------
        ys = {}
        xl2s = {}

        def p2a1(d):
            xg_d = xga[:, d * HWC:(d + 1) * HWC]
            y = p2.tile([64, HWC], F32R, tag="y", bufs=2)
            ys[d] = y
            for hf in range(2):
                ifp = p2.tile([64, 2048], F32R, tag="ifp", bufs=2)
                nc.sync.dma_start(ifp[:], inp_f[d][:, hf * 2048:(hf + 1) * 2048])
                for cq in range(4):
                    cb = 4 * hf + cq
                    sl = slice(cb * 512, (cb + 1) * 512)
                    ps3 = psB2.tile([64, 512], F32, tag="m3", bufs=3)
                    nc.tensor.matmul(ps3[:], w3Tp[:], xg_d[:, sl],
                                     start=True, stop=False,
                                     skip_group_check=True)
                    nc.tensor.matmul(
                        ps3[:], i64f, ifp[:, cq * 512:(cq + 1) * 512],
                        start=False, stop=True, skip_group_check=True)
                    nc.scalar.activation(y[:, sl], ps3[:], AF.Identity,
                                         bias=b3beta)

        def p2a2(d):
            y = ys.pop(d)
            # LN2 stats (transposed layout)
            yTs = p2.tile([128, NCH, 64], BF, tag="yTs")
            for qf in range(4):
                psT = psB2.tile([128, 512], F32R, tag="tr")
                for g in range(8):
                    cg = 8 * qf + g
                    nc.tensor.transpose(psT[:, g * 64:(g + 1) * 64],
                                        y[:, cg * 128:(cg + 1) * 128], i64f)
                nc.vector.tensor_copy(
                    yTs[:, 8 * qf:8 * (qf + 1), :],
                    psT[:].rearrange("p (g c) -> p g c", c=64))
            rv, mrv = ln_stats(yTs, "2", p2)
            rvv = rv[:].unsqueeze(2).broadcast_to([128, NCH, 64])
            mrvv = mrv[:].unsqueeze(2).broadcast_to([128, NCH, 64])
            nc.vector.tensor_mul(yTs[:], yTs[:], rvv)
            xl2 = p2.tile([128, NCH, 64], BF, tag="xl2")
            nc.vector.tensor_sub(xl2[:], yTs[:], mrvv)
            xl2s[d] = xl2

        xln2s = {}

        def p2b1(d):
            xl2 = xl2s.pop(d)
            xln2 = p2.tile([64, HWC], BF, tag="xln2")
            for hf in range(2):
                for cq in range(4):
                    psX = psB2.tile([64, 512], BF, tag="x4", bufs=1)
                    for g in range(4):
                        cg = 16 * hf + 4 * cq + g
                        nc.tensor.transpose(psX[:, g * 128:(g + 1) * 128],
                                            xl2[:, cg, :], i128)
                    nc.vector.tensor_copy(
                        xln2[:, hf * 2048 + cq * 512:
                             hf * 2048 + (cq + 1) * 512], psX[:])
            xln2s[d] = xln2

        def p2b2(d):
            # out = inp + beta*pw3 + b3beta + gamma*pw5 + b5gamma: the pw3
            # and inp terms are re-accumulated here (instead of adding the
            # staged y) so y only has to live two pipeline rounds.
            xg_d = xga[:, d * HWC:(d + 1) * HWC]
            xln2 = xln2s.pop(d)
            for hf in range(2):
                if2 = p2.tile([64, 2048], F32R, tag="if2", bufs=2)
                nc.gpsimd.dma_start(if2[:],
                                     inp_f[d][:, hf * 2048:(hf + 1) * 2048])
                outp = p2.tile([64, 2048], F32, tag="outp", bufs=2)
                xg2 = p2.tile([128, 2048], BF, tag="xg2")
                for cq in range(4):
                    cb = 4 * hf + cq
                    sl = slice(cb * 512, (cb + 1) * 512)
                    lsl = slice(cq * 512, (cq + 1) * 512)
                    ps4 = psB2.tile([128, 512], F32, tag="m45", bufs=2)
                    nc.tensor.matmul(ps4[:], w4T, xln2[:, sl],
                                     start=True, stop=True)
                    nc.scalar.activation(xg2[:, lsl], ps4[:], AF.Gelu,
                                         bias=b4)
                    ps5 = psB2.tile([64, 512], F32, tag="m45", bufs=2)
                    nc.tensor.matmul(ps5[:], w5T, xg2[:, lsl],
                                     start=True, stop=False,
                                     skip_group_check=True)
                    nc.tensor.matmul(ps5[:], w3Tp[:], xg_d[:, sl],
                                     start=False, stop=False,
                                     skip_group_check=True)
                    nc.tensor.matmul(ps5[:], i64f,
                                     if2[:, cq * 512:(cq + 1) * 512],
                                     start=False, stop=True,
                                     skip_group_check=True)
                    nc.scalar.activation(outp[:, lsl], ps5[:], AF.Identity,
                                         bias=b35)
                nc.gpsimd.dma_start(
                    out_d[d][:, hf * 2048:(hf + 1) * 2048], outp[:])

        for r in range(NPL + 3):
            if r >= 3:
                p2b2(r - 3)
            if r < NPL:
                p2a1(r)
            if r >= 1 and r - 1 < NPL:
                p2a2(r - 1)
            if r >= 2 and r - 2 < NPL:
                p2b1(r - 2)

    nc.compile()
    return nc


def _host_prep(inputs):
    inp = np.asarray(inputs["inp"], np.float32)
    style = np.asarray(inputs["style_vector"], np.float32)
    w1 = np.asarray(inputs["w1"], np.float32)
    b1 = np.asarray(inputs["b1"], np.float32)
    mod_w = np.asarray(inputs["mod_w"], np.float32)
    mod_b = np.asarray(inputs["mod_b"], np.float32)
    style_w = np.asarray(inputs["style_w"], np.float32)
    style_b = np.asarray(inputs["style_b"], np.float32)
    sca_w = np.asarray(inputs["sca_w"], np.float32)
    sca_b = np.asarray(inputs["sca_b"], np.float32)
    w3 = np.asarray(inputs["w3"], np.float32)
    b3 = np.asarray(inputs["b3"], np.float32)
    w4 = np.asarray(inputs["w4"], np.float32)
    b4 = np.asarray(inputs["b4"], np.float32)
    w5 = np.asarray(inputs["w5"], np.float32)
    b5 = np.asarray(inputs["b5"], np.float32)
    ln1_w = np.asarray(inputs["ln1_w"], np.float32).reshape(C)
    ln2_w = np.asarray(inputs["ln2_w"], np.float32).reshape(C)
    beta = np.asarray(inputs["beta"], np.float32).reshape(C)
    gamma = np.asarray(inputs["gamma"], np.float32).reshape(C)

    s = style @ style_w.T + style_b                     # [B, DW]
    k2 = (mod_w ** 2).sum(axis=(1, 2, 3, 4))            # [DW]
    demod = 1.0 / np.sqrt(k2[None] * s * s + 1e-8)      # [B, DW]
    sdv = s * demod                                     # [B, DW]

    W1t = w1 * ln1_w[None, :]                           # [DW, C]
    wdw = mod_w[:, 0]                                   # [DW, 3,3,3]

    def wtap(kind, kd, kh):
        m = np.zeros((128, 128), np.float32)
        if kind == 0:   # P: kw0 lower, kw1 upper (upper band = x shifted +1)
            m[0:64] = (W1t * wdw[:, kd, kh, 0][:, None]).T
            m[64:128] = (W1t * wdw[:, kd, kh, 1][:, None]).T
        else:           # S: kw2 lower only
            m[0:64] = (W1t * wdw[:, kd, kh, 2][:, None]).T
        return m

    wPS = np.zeros((128, 9, 2, 128), np.float32)
    for q, (t1, t2) in enumerate(DRPAIRS):
        wPS[:, q, 0] = wtap(*t1)
        wPS[:, q, 1] = wtap(*t2)

    # boundary-correction coefficients (b1 * sum of valid taps)
    def S(cd, ch, cw):
        vd = {0: [1, 2], 1: [0, 1, 2], 2: [0, 1]}[cd]
        vh = {0: [1, 2], 1: [0, 1, 2], 2: [0, 1]}[ch]
        vw = {0: [1, 2], 1: [0, 1, 2], 2: [0, 1]}[cw]
        return wdw[:, vd][:, :, vh][:, :, :, vw].sum(axis=(1, 2, 3))

    g = np.zeros((9, 64, 64), np.float32)
    g[0] = 1.0
    g[1, 0, :] = 1.0
    g[2, 63, :] = 1.0
    g[3, :, 0] = 1.0
    g[4, :, 63] = 1.0
    g[5, 0, 0] = 1.0
    g[6, 0, 63] = 1.0
    g[7, 63, 0] = 1.0
    g[8, 63, 63] = 1.0
    ind = np.zeros((9, 3, 512), np.float32)
    ind[:, 0] = g[:, 0:8, :].reshape(9, -1)
    ind[:, 1] = g[:, 8:16, :].reshape(9, -1)
    ind[:, 2] = g[:, 56:64, :].reshape(9, -1)

    def corr_for(dcase):
        c = np.zeros((9, 128), np.float32)
        base = S(dcase, 1, 1)
        c[0] = base
        c[1] = S(dcase, 0, 1) - base
        c[2] = S(dcase, 2, 1) - base
        c[3] = S(dcase, 1, 0) - base
        c[4] = S(dcase, 1, 2) - base
        c[5] = S(dcase, 0, 0) - S(dcase, 0, 1) - S(dcase, 1, 0) + base
        c[6] = S(dcase, 0, 2) - S(dcase, 0, 1) - S(dcase, 1, 2) + base
        c[7] = S(dcase, 2, 0) - S(dcase, 2, 1) - S(dcase, 1, 0) + base
        c[8] = S(dcase, 2, 2) - S(dcase, 2, 1) - S(dcase, 1, 2) + base
        return c * b1[None, :]

    corr_tab = {c: corr_for(c) for c in (0, 1, 2)}

    blob8 = np.zeros((128, 8064), f8)
    blob8[:, 0:2304] = (wPS * S_W).astype(f8).reshape(128, 2304)
    blob8[:, 2304:2432] = np.eye(128, dtype=np.float32).astype(f8)
    blob8[0:9, 6528:8064] = ind.astype(f8).reshape(9, 1536)
    blobb = np.zeros((128, 512), bf)
    blobb[:, 0:64] = (w3.T * beta[None, :]).astype(bf)
    blobb[:, 64:192] = (sca_w.T / float(D * H * W)).astype(bf)
    blobb[0:64, 192:320] = (w4 * ln2_w[None, :]).T.astype(bf)
    blobb[:, 320:384] = (w5.T * gamma[None, :]).astype(bf)
    blobb[:, 384:512] = np.eye(128, dtype=np.float32).astype(bf)
    blobf = np.zeros((128, 70), np.float32)
    blobf[:, 1] = mod_b
    blobf[:, 2] = sca_b
    blobf[0:64, 3] = b3 * beta
    blobf[:, 4] = b4
    blobf[0:64, 5] = b5 * gamma
    blobf[0:64, 6] = b3 * beta + b5 * gamma
    common = dict(blob8=blob8, blobb=blobb,
                  i64f=np.eye(64, dtype=np.float32))

    in_maps = []
    for k in range(8):
        b, d0 = k // 4, (k % 4) * NPL
        ip = inp[b]
        halo = np.zeros((NHALO, C, HWC), np.float32)
        lo, hi = max(d0 - 1, 0), min(d0 + NPL + 1, D)
        halo[lo - (d0 - 1):hi - (d0 - 1)] = (
            ip[:, lo:hi].transpose(1, 0, 2, 3).reshape(hi - lo, C, HWC))
        wcorr = np.zeros((9, NPL, 2, 128), np.float32)
        for i in range(NPL):
            dg = d0 + i
            dcase = 0 if dg == 0 else (2 if dg == D - 1 else 1)
            wcorr[:, i, 0, :] = corr_tab[dcase]
        m = dict(common)
        m["inp_t"] = halo.astype(bf)
        m["inp_f"] = np.ascontiguousarray(
            ip[:, d0:d0 + NPL].transpose(1, 0, 2, 3).reshape(NPL, C, HWC))
        b8 = blob8.copy()
        b8[0:9, 2432:6528] = (wcorr * S_W).astype(f8).reshape(9, 4096)
        m["blob8"] = b8
        bff = blobf.copy()
        bff[:, 0] = (sdv[b] / S_W)
        m["blobf"] = bff
        in_maps.append(m)
    return in_maps


def kernel(**inputs):
    from concourse.bass_utils import run_bass_kernel_spmd
    if "nc" not in _CACHE:
        _CACHE["nc"] = _build()
    nc = _CACHE["nc"]
    in_maps = _host_prep(inputs)
    res = run_bass_kernel_spmd(nc, in_maps, list(range(8)))
    _CACHE["last_res"] = res
    out = np.empty((2, C, D, H, W), np.float32)
    for k in range(8):
        b, d0 = k // 4, (k % 4) * NPL
        o = res.results[k]["out"]
        out[b, :, d0:d0 + NPL] = o.reshape(NPL, C, H, W).transpose(1, 0, 2, 3)
    return out


# revision 6
# speedup vs baseline: 1.0369x; 1.0198x over previous
"""Trainium2 Bass kernel for nn_BaselineBlock_SCA_Modulated — v2.

Sharding: 8 cores = 2 batch x 4 D-slabs of 16 planes. Halo planes staged
host-side (zeros at global D edges) so all cores run one SPMD program.

v2 changes vs baseline:
- Conv chain in fp8 (e4m3) with DoubleRow matmuls: 27 taps + boundary corr
  in 11 matmuls / 7 N-units per 512-chunk (vs 16 bf16 matmuls).
- Gelu output (xg) stays resident in SBUF as fp8 (no DRAM round trip).
- LN stats via bn_stats; dual-shift ring copies via uint16-bitcast 2x copies.
- Residual adds done on PE (identity f32r matmuls into PSUM) with Act
  readout, instead of DVE affine ops.
- DMA queues split: loads on SP, stores on Pool.
"""
import numpy as np
import ml_dtypes

C, DW, SD = 64, 128, 512
D, H, W = 64, 64, 64
NPL = 16              # output planes per core
NHALO = NPL + 2       # input planes incl halo
PW = 68               # padded row width (2 left pad + 64 + 2 right pad)
NPR = 66              # padded row count (1 top + 64 + 1 bottom)
PSZ = NPR * PW        # padded plane size (even)
HWC = H * W           # 4096
NCH = 32              # 128-position chunks per plane
EPS = 1e-6
S_W = 256.0           # fp8 weight prescale
bf = ml_dtypes.bfloat16
f8 = ml_dtypes.float8_e4m3fn

# (kind, kd, kh): kind 0 = P-read (kw0 lower / kw1 upper, col 1),
# kind 1 = S-read (kw2 lower, zero upper, col 3)
TAPS = [(0, kd, kh) for kd in range(3) for kh in range(3)] + \
       [(1, kd, kh) for kd in range(3) for kh in range(3)]
DRPAIRS = [(TAPS[2 * i], TAPS[2 * i + 1]) for i in range(9)]

_CACHE = {}


def _build():
    import concourse.bacc as bacc
    import concourse.mybir as mybir
    import concourse.tile as tile
    from concourse.ap import AP
    from concourse.mybir import ActivationFunctionType as AF, AluOpType as ALU

    BF = mybir.dt.bfloat16
    F32 = mybir.dt.float32
    F32R = mybir.dt.float32r
    FP8 = mybir.dt.float8e4
    U16 = mybir.dt.uint16
    AX = mybir.AxisListType
    DR = mybir.MatmulPerfMode.DoubleRow

    nc = bacc.Bacc("TRN2", target_bir_lowering=False, debug=False, num_devices=8)

    dram = {}
    def din(name, shape, dt=BF):
        dram[name] = nc.dram_tensor(name, shape, dt, kind="ExternalInput")
        return dram[name]

    inp_t = din("inp_t", [NHALO, C, HWC], BF)
    inp_f = din("inp_f", [NPL, C, HWC], F32R)
    blob8_i = din("blob8", [128, 8064], FP8)
    blobb_i = din("blobb", [128, 512], BF)
    blobf_i = din("blobf", [128, 70], F32)
    i64f_i = din("i64f", [64, 64], F32R)
    out_d = nc.dram_tensor("out", [NPL, C, HWC], F32, kind="ExternalOutput")

    cc_a = nc.dram_tensor("cc_a", [128, 1], F32)
    cc_b = nc.dram_tensor("cc_b", [128, 1], F32)

    from contextlib import ExitStack
    with tile.TileContext(nc) as tc, ExitStack() as stk:
        cpool = stk.enter_context(tc.tile_pool(name="const", bufs=1))
        sm = stk.enter_context(tc.tile_pool(name="small", bufs=3))
        p1stk = ExitStack()
        wp1 = p1stk.enter_context(tc.tile_pool(name="p1", bufs=2))
        rpool = p1stk.enter_context(tc.tile_pool(name="ring", bufs=1))
        psC = p1stk.enter_context(tc.tile_pool(name="psC", bufs=4,
                                               space="PSUM"))

        def const(name, shape, dt):
            t = cpool.tile(shape, dt, tag=name, name=name)
            nc.sync.dma_start(t[:], dram[name][:])
            return t

        t8 = const("blob8", [128, 8064], FP8)
        tb = const("blobb", [128, 512], BF)
        tf = const("blobf", [128, 70], F32)
        i64t = const("i64f", [64, 64], F32R)
        wPS = t8[:, 0:2304].rearrange("p (q t m) -> p q t m", q=9, t=2)
        i128f8 = t8[:, 2304:2432]
        wc = t8[0:9, 2432:6528].rearrange("p (d t m) -> p d t m", d=16, t=2)
        ind = t8[0:9, 6528:8064].rearrange("p (a b) -> p a b", a=3)
        w3T = tb[:, 0:64]
        scawT = tb[:, 64:192]
        w4T = tb[0:64, 192:320]
        w5T = tb[:, 320:384]
        i128 = tb[:, 384:512]
        sd = tf[:, 0:1]
        modb = tf[:, 1:2]
        scab = tf[:, 2:3]
        b3beta = tf[0:64, 3:4]
        b4 = tf[:, 4:5]
        b5g = tf[0:64, 5:6]
        b35 = tf[0:64, 6:7]
        i64f = i64t[:]

        pools = cpool.tile([128, NPL * 8], F32, tag="pools")
        w3Tp = cpool.tile([128, 64], BF, tag="w3Tp")
        # persistent ring (4 slots in ONE tensor for cross-slot DoubleRow
        # k-tile reads). fp8 values live at byte stride 2 (the fp8 PE
        # transpose writes with element step 2), so the ring is a U16 tile
        # whose low... each u16 cell holds one fp8 value; u16 copies move it.
        ring = rpool.tile([128, 4 * PSZ], U16, tag="ring", name="ring")
        xga = cpool.tile([128, NPL * HWC], FP8, tag="xga", name="xga")
        nc.gpsimd.memset(ring[:], 0)

        rAP = ring[:]
        ring_pstride = rAP.ap[0][0]
        r8 = rAP.bitcast(FP8)

        def rd(parts, offv, dims):
            """fp8 AP into the ring; offv/dims in VALUE units (1 value =
            2 bytes = 2 fp8 elements; strides passed here get doubled)."""
            return AP(r8.tensor, r8.offset + 2 * offv,
                      [[r8.ap[0][0], parts]] + [[2 * s, n] for s, n in dims])

        # ---------------- PASS 1 ----------------
        def ln_stats(xT, sfx, pool):
            """per-(partition,chunk) LN stats over the 64-ch innermost dim.
            Returns (rv, mrv) [128, NCH] f32."""
            sq = pool.tile([128, NCH, 64], BF, tag="sq" + sfx, bufs=1)
            nc.vector.tensor_mul(sq[:], xT[:], xT[:])
            msum = sm.tile([128, NCH], BF, tag="msum" + sfx)
            qsum = sm.tile([128, NCH], BF, tag="qsum" + sfx)
            with nc.allow_low_precision(reason="stat sums; dve accumulates "
                                        "fp32 internally, bf16 round-off is "
                                        "far below the fp8 conv noise"):
                nc.vector.tensor_reduce(msum[:], xT[:], axis=AX.X, op=ALU.add)
                nc.vector.tensor_reduce(qsum[:], sq[:], axis=AX.X, op=ALU.add)
            t1v = sm.tile([128, NCH], F32, tag="t1v" + sfx)
            nc.vector.tensor_mul(t1v[:], msum[:], msum[:])
            t3v = sm.tile([128, NCH], F32, tag="t3v" + sfx)
            nc.vector.tensor_scalar_mul(t3v[:], qsum[:], 1.0 / 63.0)
            var = sm.tile([128, NCH], F32, tag="var" + sfx)
            nc.vector.scalar_tensor_tensor(
                var[:], t1v[:], -1.0 / (64.0 * 63.0), t3v[:],
                op0=ALU.mult, op1=ALU.add)
            # rsqrt(var) via Newton with r0 = 2/(1+v): a global underestimate
            # of rsqrt (AM-GM), so iteration converges monotonically; keeps
            # ACT parked on the gelu table set. eps (1e-6 on std ~1) is far
            # below fp8/bf16 noise and is dropped.
            sv = sm.tile([128, NCH], F32, tag="sv" + sfx)
            nc.vector.tensor_scalar(sv[:], var[:], 0.5, 0.5,
                                    op0=ALU.mult, op1=ALU.add)
            rv = sm.tile([128, NCH], F32, tag="rv" + sfx)
            nc.vector.reciprocal(rv[:], sv[:])
            tq = sm.tile([128, NCH], F32, tag="tq" + sfx)
            for _ in range(2):
                nc.vector.tensor_mul(tq[:], rv[:], rv[:])
                nc.vector.tensor_mul(tq[:], tq[:], var[:])
                nc.vector.tensor_scalar(tq[:], tq[:], -0.5, 1.5,
                                        op0=ALU.mult, op1=ALU.add)
                nc.vector.tensor_mul(rv[:], rv[:], tq[:])
            mrv = sm.tile([128, NCH], F32, tag="mrv" + sfx)
            nc.vector.scalar_tensor_tensor(
                mrv[:], msum[:], 1.0 / 64.0, rv[:], op0=ALU.mult, op1=ALU.mult)
            return rv, mrv

        xl8s = {}

        def ln1a(p):
            xT = wp1.tile([128, NCH, 64], BF, tag="xT", bufs=3)
            nc.sync.dma_start_transpose(xT[:], inp_t[p])
            rv, mrv = ln_stats(xT, "1", wp1)
            # apply LN (mul in place, sub quantizes to fp8)
            rvv = rv[:].unsqueeze(2).broadcast_to([128, NCH, 64])
            mrvv = mrv[:].unsqueeze(2).broadcast_to([128, NCH, 64])
            nc.vector.tensor_mul(xT[:], xT[:], rvv)
            xl8 = wp1.tile([128, NCH, 64], FP8, tag="xl8", bufs=3)
            nc.vector.tensor_sub(xl8[:], xT[:], mrvv)
            xl8s[p] = xl8

        def ln1b(p):
            slot = p % 4
            xl8 = xl8s.pop(p)
            # transpose to ring layout: fp8 transposes write with element
            # step 2 (hw requirement) into psF; u16 copies move value cells.
            for hf in range(2):
                psF = psC.tile([64, HWC], FP8, tag="trF", bufs=2)
                for g in range(16):
                    dst = psF[:, g * 256:(g + 1) * 256].rearrange(
                        "p (c two) -> p c two", two=2)[:, :, 0]
                    nc.tensor.transpose(dst, xl8[:, 16 * hf + g, :], i128f8)
                srcu = psF[:].bitcast(U16).rearrange(
                    "p (r w) -> p r w", w=64)
                o_lo = slot * PSZ + (1 + 32 * hf) * PW + 2
                dst_lo = AP(rAP.tensor, rAP.offset + o_lo,
                            [[ring_pstride, 64], [PW, 32], [1, 64]])
                nc.scalar.copy(dst_lo, srcu)
            # upper dual band (x shifted +1 value) via one contiguous
            # Pool-issued DMA over the full interior row block
            src_full = AP(rAP.tensor, rAP.offset + slot * PSZ + PW,
                          [[ring_pstride, 64], [1, 64 * PW]])
            dst_full = AP(rAP.tensor,
                          rAP.offset + 64 * ring_pstride + slot * PSZ + PW - 1,
                          [[ring_pstride, 64], [1, 64 * PW]])
            nc.gpsimd.dma_start(dst_full, src_full)

        def aoff(slots, kd, kh, cb):
            return slots[kd] * PSZ + (8 * cb + kh) * PW

        def conv_plane(d):
            slots = [(d + kd) % 4 for kd in range(3)]
            pat = lambda cb: 0 if cb == 0 else (2 if cb == 7 else 1)
            for cb in range(8):
                ps = psC.tile([128, 512], F32, tag="mm", bufs=4)
                first = True
                for q, (t1, t2) in enumerate(DRPAIRS):
                    o1 = aoff(slots, t1[1], t1[2], cb) + 1 + 2 * t1[0]
                    o2 = aoff(slots, t2[1], t2[2], cb) + 1 + 2 * t2[0]
                    rhs = rd(128, o1, [[o2 - o1, 2], [PW, 8], [1, 64]])
                    nc.tensor.matmul(ps[:], wPS[:, q], rhs, start=first,
                                     stop=False, perf_mode=DR,
                                     skip_group_check=True)
                    first = False
                rhs_c = ind[:, pat(cb), :].unsqueeze(1).broadcast_to(
                    [9, 2, 512])
                nc.tensor.matmul(ps[:], wc[:, d], rhs_c, start=False,
                                 stop=True, perf_mode=DR,
                                 skip_group_check=True)
                nc.scalar.activation(
                    xga[:, d * HWC + cb * 512: d * HWC + (cb + 1) * 512],
                    ps[:], AF.Gelu, bias=modb, scale=sd,
                    accum_out=pools[:, d * 8 + cb:d * 8 + cb + 1])

        for r in range(NHALO + 2):
            if r >= 4:
                conv_plane(r - 4)
            if r >= 1 and r - 1 < NHALO:
                ln1b(r - 1)
            if r < NHALO:
                ln1a(r)

        p1stk.close()
        p2 = stk.enter_context(tc.tile_pool(name="p2", bufs=2))
        psB2 = stk.enter_context(tc.tile_pool(name="psB2", bufs=2,
                                              space="PSUM"))

        # ---------------- pooled -> gate ----------------
        pooled = cpool.tile([128, 1], F32, tag="pooled")
        nc.vector.tensor_reduce(pooled[:], pools[:], axis=AX.X, op=ALU.add)
        nc.gpsimd.dma_start(cc_a[:], pooled[:])
        nc.gpsimd.collective_compute(
            "AllReduce", ALU.add,
            replica_groups=[[0, 1, 2, 3], [4, 5, 6, 7]],
            ins=[cc_a[:]], outs=[cc_b[:]])
        pooled2f = cpool.tile([128, 1], F32, tag="pooled2f", name="pooled2f")
        nc.gpsimd.dma_start(pooled2f[:], cc_b[:])
        pooled2 = cpool.tile([128, 1], BF, tag="pooled2", name="pooled2")
        nc.vector.tensor_copy(pooled2[:], pooled2f[:])
        psg = psB2.tile([128, 512], F32, tag="m45", bufs=2)
        nc.tensor.matmul(psg[:, 0:1], scawT, pooled2[:], start=True,
                         stop=True)
        gate = cpool.tile([128, 1], F32, tag="gatev")
        nc.scalar.activation(gate[:], psg[:, 0:1], AF.Identity, bias=scab)
        nc.vector.tensor_scalar_mul(w3Tp[:], w3T, gate[:])

        # ---------------- PASS 2 ----------------
        ys = {}
        xl2s = {}

        def p2a1(d):
            xg_d = xga[:, d * HWC:(d + 1) * HWC]
            y = p2.tile([64, HWC], F32R, tag="y", bufs=2)
            ys[d] = y
            for hf in range(2):
                ifp = p2.tile([64, 2048], F32R, tag="ifp", bufs=2)
                nc.sync.dma_start(ifp[:], inp_f[d][:, hf * 2048:(hf + 1) * 2048])
                for cq in range(4):
                    cb = 4 * hf + cq
                    sl = slice(cb * 512, (cb + 1) * 512)
                    ps3 = psB2.tile([64, 512], F32, tag="m3", bufs=2)
                    nc.tensor.matmul(ps3[:], w3Tp[:], xg_d[:, sl],
                                     start=True, stop=False,
                                     skip_group_check=True)
                    nc.tensor.matmul(
                        ps3[:], i64f, ifp[:, cq * 512:(cq + 1) * 512],
                        start=False, stop=True, skip_group_check=True)
                    nc.scalar.activation(y[:, sl], ps3[:], AF.Identity,
                                         bias=b3beta)

        def p2a2(d):
            y = ys.pop(d)
            # LN2 stats (transposed layout)
            yTs = p2.tile([128, NCH, 64], BF, tag="yTs")
            for qf in range(4):
                psT = psB2.tile([128, 512], F32R, tag="tr", bufs=3)
                for g in range(8):
                    cg = 8 * qf + g
                    nc.tensor.transpose(psT[:, g * 64:(g + 1) * 64],
                                        y[:, cg * 128:(cg + 1) * 128], i64f)
                nc.vector.tensor_copy(
                    yTs[:, 8 * qf:8 * (qf + 1), :],
                    psT[:].rearrange("p (g c) -> p g c", c=64))
            rv, mrv = ln_stats(yTs, "2", p2)
            rvv = rv[:].unsqueeze(2).broadcast_to([128, NCH, 64])
            mrvv = mrv[:].unsqueeze(2).broadcast_to([128, NCH, 64])
            nc.vector.tensor_mul(yTs[:], yTs[:], rvv)
            xl2 = p2.tile([128, NCH, 64], BF, tag="xl2")
            nc.vector.tensor_sub(xl2[:], yTs[:], mrvv)
            xl2s[d] = xl2

        xln2s = {}

        def p2b1(d):
            xl2 = xl2s.pop(d)
            xln2 = p2.tile([64, HWC], BF, tag="xln2")
            for hf in range(2):
                for cq in range(4):
                    psX = psB2.tile([64, 512], BF, tag="x4", bufs=1)
                    for g in range(4):
                        cg = 16 * hf + 4 * cq + g
                        nc.tensor.transpose(psX[:, g * 128:(g + 1) * 128],
                                            xl2[:, cg, :], i128)
                    nc.vector.tensor_copy(
                        xln2[:, hf * 2048 + cq * 512:
                             hf * 2048 + (cq + 1) * 512], psX[:])
            xln2s[d] = xln2

        def p2b2(d):
            # out = inp + beta*pw3 + b3beta + gamma*pw5 + b5gamma: the pw3
            # and inp terms are re-accumulated here (instead of adding the
            # staged y) so y only has to live two pipeline rounds.
            xg_d = xga[:, d * HWC:(d + 1) * HWC]
            xln2 = xln2s.pop(d)
            for hf in range(2):
                if2 = p2.tile([64, 2048], F32R, tag="if2", bufs=2)
                nc.gpsimd.dma_start(if2[:],
                                     inp_f[d][:, hf * 2048:(hf + 1) * 2048])
                outp = p2.tile([64, 2048], F32, tag="outp", bufs=2)
                xg2 = p2.tile([128, 2048], BF, tag="xg2")
                for cq in range(4):
                    cb = 4 * hf + cq
                    sl = slice(cb * 512, (cb + 1) * 512)
                    lsl = slice(cq * 512, (cq + 1) * 512)
                    ps4 = psB2.tile([128, 512], F32, tag="m45", bufs=2)
                    nc.tensor.matmul(ps4[:], w4T, xln2[:, sl],
                                     start=True, stop=True)
                    nc.scalar.activation(xg2[:, lsl], ps4[:], AF.Gelu,
                                         bias=b4)
                    ps5 = psB2.tile([64, 512], F32, tag="m45", bufs=2)
                    nc.tensor.matmul(ps5[:], w5T, xg2[:, lsl],
                                     start=True, stop=False,
                                     skip_group_check=True)
                    nc.tensor.matmul(ps5[:], w3Tp[:], xg_d[:, sl],
                                     start=False, stop=False,
                                     skip_group_check=True)
                    nc.tensor.matmul(ps5[:], i64f,
                                     if2[:, cq * 512:(cq + 1) * 512],
                                     start=False, stop=True,
                                     skip_group_check=True)
                    nc.scalar.activation(outp[:, lsl], ps5[:], AF.Identity,
                                         bias=b35)
                nc.gpsimd.dma_start(
                    out_d[d][:, hf * 2048:(hf + 1) * 2048], outp[:])

        for r in range(NPL + 3):
            if r >= 3:
                p2b2(r - 3)
            if r < NPL:
                p2a1(r)
            if r >= 1 and r - 1 < NPL:
                p2a2(r - 1)
            if r >= 2 and r - 2 < NPL:
                p2b1(r - 2)

    nc.compile()
    return nc


def _host_prep(inputs):
    inp = np.asarray(inputs["inp"], np.float32)
    style = np.asarray(inputs["style_vector"], np.float32)
    w1 = np.asarray(inputs["w1"], np.float32)
    b1 = np.asarray(inputs["b1"], np.float32)
    mod_w = np.asarray(inputs["mod_w"], np.float32)
    mod_b = np.asarray(inputs["mod_b"], np.float32)
    style_w = np.asarray(inputs["style_w"], np.float32)
    style_b = np.asarray(inputs["style_b"], np.float32)
    sca_w = np.asarray(inputs["sca_w"], np.float32)
    sca_b = np.asarray(inputs["sca_b"], np.float32)
    w3 = np.asarray(inputs["w3"], np.float32)
    b3 = np.asarray(inputs["b3"], np.float32)
    w4 = np.asarray(inputs["w4"], np.float32)
    b4 = np.asarray(inputs["b4"], np.float32)
    w5 = np.asarray(inputs["w5"], np.float32)
    b5 = np.asarray(inputs["b5"], np.float32)
    ln1_w = np.asarray(inputs["ln1_w"], np.float32).reshape(C)
    ln2_w = np.asarray(inputs["ln2_w"], np.float32).reshape(C)
    beta = np.asarray(inputs["beta"], np.float32).reshape(C)
    gamma = np.asarray(inputs["gamma"], np.float32).reshape(C)

    s = style @ style_w.T + style_b                     # [B, DW]
    k2 = (mod_w ** 2).sum(axis=(1, 2, 3, 4))            # [DW]
    demod = 1.0 / np.sqrt(k2[None] * s * s + 1e-8)      # [B, DW]
    sdv = s * demod                                     # [B, DW]

    W1t = w1 * ln1_w[None, :]                           # [DW, C]
    wdw = mod_w[:, 0]                                   # [DW, 3,3,3]

    def wtap(kind, kd, kh):
        m = np.zeros((128, 128), np.float32)
        if kind == 0:   # P: kw0 lower, kw1 upper (upper band = x shifted +1)
            m[0:64] = (W1t * wdw[:, kd, kh, 0][:, None]).T
            m[64:128] = (W1t * wdw[:, kd, kh, 1][:, None]).T
        else:           # S: kw2 lower only
            m[0:64] = (W1t * wdw[:, kd, kh, 2][:, None]).T
        return m

    wPS = np.zeros((128, 9, 2, 128), np.float32)
    for q, (t1, t2) in enumerate(DRPAIRS):
        wPS[:, q, 0] = wtap(*t1)
        wPS[:, q, 1] = wtap(*t2)

    # boundary-correction coefficients (b1 * sum of valid taps)
    def S(cd, ch, cw):
        vd = {0: [1, 2], 1: [0, 1, 2], 2: [0, 1]}[cd]
        vh = {0: [1, 2], 1: [0, 1, 2], 2: [0, 1]}[ch]
        vw = {0: [1, 2], 1: [0, 1, 2], 2: [0, 1]}[cw]
        return wdw[:, vd][:, :, vh][:, :, :, vw].sum(axis=(1, 2, 3))

    g = np.zeros((9, 64, 64), np.float32)
    g[0] = 1.0
    g[1, 0, :] = 1.0
    g[2, 63, :] = 1.0
    g[3, :, 0] = 1.0
    g[4, :, 63] = 1.0
    g[5, 0, 0] = 1.0
    g[6, 0, 63] = 1.0
    g[7, 63, 0] = 1.0
    g[8, 63, 63] = 1.0
    ind = np.zeros((9, 3, 512), np.float32)
    ind[:, 0] = g[:, 0:8, :].reshape(9, -1)
    ind[:, 1] = g[:, 8:16, :].reshape(9, -1)
    ind[:, 2] = g[:, 56:64, :].reshape(9, -1)

    def corr_for(dcase):
        c = np.zeros((9, 128), np.float32)
        base = S(dcase, 1, 1)
        c[0] = base
        c[1] = S(dcase, 0, 1) - base
        c[2] = S(dcase, 2, 1) - base
        c[3] = S(dcase, 1, 0) - base
        c[4] = S(dcase, 1, 2) - base
        c[5] = S(dcase, 0, 0) - S(dcase, 0, 1) - S(dcase, 1, 0) + base
        c[6] = S(dcase, 0, 2) - S(dcase, 0, 1) - S(dcase, 1, 2) + base
        c[7] = S(dcase, 2, 0) - S(dcase, 2, 1) - S(dcase, 1, 0) + base
        c[8] = S(dcase, 2, 2) - S(dcase, 2, 1) - S(dcase, 1, 2) + base
        return c * b1[None, :]

    corr_tab = {c: corr_for(c) for c in (0, 1, 2)}

    blob8 = np.zeros((128, 8064), f8)
    blob8[:, 0:2304] = (wPS * S_W).astype(f8).reshape(128, 2304)
    blob8[:, 2304:2432] = np.eye(128, dtype=np.float32).astype(f8)
    blob8[0:9, 6528:8064] = ind.astype(f8).reshape(9, 1536)
    blobb = np.zeros((128, 512), bf)
    blobb[:, 0:64] = (w3.T * beta[None, :]).astype(bf)
    blobb[:, 64:192] = (sca_w.T / float(D * H * W)).astype(bf)
    blobb[0:64, 192:320] = (w4 * ln2_w[None, :]).T.astype(bf)
    blobb[:, 320:384] = (w5.T * gamma[None, :]).astype(bf)
    blobb[:, 384:512] = np.eye(128, dtype=np.float32).astype(bf)
    blobf = np.zeros((128, 70), np.float32)
    blobf[:, 1] = mod_b
    blobf[:, 2] = sca_b
    blobf[0:64, 3] = b3 * beta
    blobf[:, 4] = b4
    blobf[0:64, 5] = b5 * gamma
    blobf[0:64, 6] = b3 * beta + b5 * gamma
    common = dict(blob8=blob8, blobb=blobb,
                  i64f=np.eye(64, dtype=np.float32))

    in_maps = []
    for k in range(8):
        b, d0 = k // 4, (k % 4) * NPL
        ip = inp[b]
        halo = np.zeros((NHALO, C, HWC), np.float32)
        lo, hi = max(d0 - 1, 0), min(d0 + NPL + 1, D)
        halo[lo - (d0 - 1):hi - (d0 - 1)] = (
            ip[:, lo:hi].transpose(1, 0, 2, 3).reshape(hi - lo, C, HWC))
        wcorr = np.zeros((9, NPL, 2, 128), np.float32)
        for i in range(NPL):
            dg = d0 + i
            dcase = 0 if dg == 0 else (2 if dg == D - 1 else 1)
            wcorr[:, i, 0, :] = corr_tab[dcase]
        m = dict(common)
        m["inp_t"] = halo.astype(bf)
        m["inp_f"] = np.ascontiguousarray(
            ip[:, d0:d0 + NPL].transpose(1, 0, 2, 3).reshape(NPL, C, HWC))
        b8 = blob8.copy()
        b8[0:9, 2432:6528] = (wcorr * S_W).astype(f8).reshape(9, 4096)
        m["blob8"] = b8
        bff = blobf.copy()
        bff[:, 0] = (sdv[b] / S_W)
        m["blobf"] = bff
        in_maps.append(m)
    return in_maps


def kernel(**inputs):
    from concourse.bass_utils import run_bass_kernel_spmd
    if "nc" not in _CACHE:
        _CACHE["nc"] = _build()
    nc = _CACHE["nc"]
    in_maps = _host_prep(inputs)
    res = run_bass_kernel_spmd(nc, in_maps, list(range(8)))
    _CACHE["last_res"] = res
    out = np.empty((2, C, D, H, W), np.float32)
    for k in range(8):
        b, d0 = k // 4, (k % 4) * NPL
        o = res.results[k]["out"]
        out[b, :, d0:d0 + NPL] = o.reshape(NPL, C, H, W).transpose(1, 0, 2, 3)
    return out
